# revision 1
# baseline (speedup 1.0000x reference)
"""Grouped-Query Attention kernel for 8 Trainium2 NeuronCores.

Reference model: x[1,2048,2048] -> Q(32 heads x 64) / K,V(8 kv heads x 64),
per-head RMS-norm(Q,K) + RoPE, causal softmax attention, out-projection.

Sharding (tensor-parallel over heads): core c owns Q heads 4c..4c+3 and KV
head c (exactly its GQA group) and W_out rows [256c : 256c+256).  Each core
computes a full-shape partial output; the host sums the 8 partials (the
unshard step for a row-sharded W_out).

On-core strategy:
  - all matmul operands are bf16 (PE runs 1 cycle/row vs 4 for fp32);
    accumulation stays fp32 in PSUM, softmax statistics stay fp32
  - x and the weights are converted to bf16 on the host, so DMA traffic
    is halved and no on-chip conversion pass is needed
  - scores are built TRANSPOSED (S^T[j,i] = k_j . q_i) so PV needs no
    attention-matrix transpose and the softmax denominator comes free
    via an extra ones-column in V
  - RMS-norm of q/k bounds |scores/8| <= 8, so exp() without
    max-subtraction is safe; masked entries are zeroed after exp by
    multiplying with precomputed keep-masks (causal edge tiles dedupe
    to 4 patterns)
  - phase 2 runs i-block outer so denominators + out-projection for
    early token blocks overlap later attention; per (head, iblock) all
    QK matmuls issue before the PV accumulation so exp latency hides
  - q/k norm+rope are batched as 5 "heads" (4 q + 1 k) with the q/k
    scales and rotate-half signs folded into host-precomputed tables
"""

import numpy as np
import ml_dtypes

BF16 = ml_dtypes.bfloat16

T = 2048
D = 2048
NUM_HEADS = 32
NUM_KV = 8
HD = 64
N_CORES = 8
H_LOC = NUM_HEADS // N_CORES  # 4 q heads per core
EPS = 1e-6

TT = T // 128   # 16 t-tiles of 128 rows
CC = D // 128   # 16 contraction chunks
IBS = T // 512  # 4 i-blocks of 512 query positions
JBS = T // 128  # 16 j-blocks of 128 key positions

KEEP = "keep"
SKIP = "skip"
AFFINE = "affine"  # kept for test.py compat; no longer produced


def _classify_mask(mask: np.ndarray):
    """Per (ib, jb) scoresT tile: KEEP / SKIP / ('pat', idx) with deduped
    multiplicative keep-masks in S^T (j, i) layout.  A causal mask yields
    just 4 distinct edge patterns."""
    keep = ~mask
    status = [[KEEP] * JBS for _ in range(IBS)]
    pat_index: dict[bytes, int] = {}
    pats: list[np.ndarray] = []
    for ib in range(IBS):
        for jb in range(JBS):
            sub = keep[ib * 512:(ib + 1) * 512, jb * 128:(jb + 1) * 128]
            if sub.all():
                status[ib][jb] = KEEP
            elif not sub.any():
                status[ib][jb] = SKIP
            else:
                key = sub.tobytes()
                if key not in pat_index:
                    pat_index[key] = len(pats)
                    pats.append(sub.T.astype(np.float32))  # [128 j, 512 i]
                status[ib][jb] = ("pat", pat_index[key])
    patterns = (
        np.stack(pats) if pats else np.zeros((1, 128, 512), dtype=np.float32)
    )
    # leading i-columns that are fully masked in each pattern: the score
    # matmul / exp / PV only need the live suffix
    prefixes = []
    for p in patterns:
        alive = p.any(axis=0)
        prefixes.append(int(alive.argmax()) if alive.any() else 512)
    return status, patterns, prefixes


def _split_multiwaits(nc):
    """walrus in this container accepts only ONE sync-wait per instruction;
    hoist extra waits onto preceding same-engine NoOps (program order on the
    engine queue preserves the gating)."""
    import bass_rust
    from concourse import mybir

    n_fixed = 0
    for fn in nc.m.functions:
        for bb in fn.blocks:
            out = []
            for ins in bb.instructions:
                si = ins.sync_info
                if si is not None and si.on_wait and len(si.on_wait) > 1:
                    waits = list(si.on_wait)
                    ups = list(si.on_update) if si.on_update else []
                    for k, w in enumerate(waits[:-1]):
                        nop = mybir.InstNoOp(
                            name=f"{ins.name}-wnop{k}", ins=[], outs=[]
                        )
                        nop.engine = ins.engine
                        nop.sync_info = bass_rust.SyncInfo(
                            on_wait=[w], on_update=[]
                        )
                        out.append(nop)
                    ins.sync_info = bass_rust.SyncInfo(
                        on_wait=[waits[-1]], on_update=ups
                    )
                    n_fixed += 1
                out.append(ins)
            bb.instructions = out
    return n_fixed


def _build_program(status, n_pat, pat_prefix):
    import concourse.bass as bass
    import concourse.mybir as mybir
    import concourse.tile as tile
    from concourse.masks import make_identity

    f32 = mybir.dt.float32
    bf16 = mybir.dt.bfloat16
    AX = mybir.AxisListType
    AF = mybir.ActivationFunctionType

    nc = bass.Bass("TRN2", num_devices=N_CORES)
    x_d = nc.declare_dram_parameter("x", [T, D], bf16, isOutput=False)
    wqkv_d = nc.declare_dram_parameter(
        "wqkv", [D, H_LOC * HD + 2 * HD], bf16, isOutput=False
    )
    wo_d = nc.declare_dram_parameter("wo", [H_LOC * HD, D], bf16, isOutput=False)
    # combined 5-unit rope tables: 4 q heads + k, scales folded in
    cosa_d = nc.declare_dram_parameter("cosa", [T, 5 * HD], bf16, isOutput=False)
    sina_d = nc.declare_dram_parameter("sina", [T, 5 * HD], bf16, isOutput=False)
    mpat_d = nc.declare_dram_parameter(
        "mpat", [n_pat, 128, 512], bf16, isOutput=False
    )
    out_d = nc.declare_dram_parameter("out", [T, D], bf16, isOutput=True)

    NQKV = H_LOC * HD + 2 * HD  # 384: q heads, then k, then v
    NQK = (H_LOC + 1) * HD      # 320: q heads + k (norm/rope batch)

    def mmr(out, lhsT, rhs, **kw):
        nc.tensor.matmul(out, lhsT, rhs, **kw)

    with tile.TileContext(nc) as tc:
        with (
            tc.tile_pool(name="const", bufs=1) as const,
            tc.tile_pool(name="persist", bufs=1) as persist,
        ):
            ident = const.tile([128, 128], bf16)
            make_identity(nc, ident)
            eps_t = const.tile([128, 1], f32)
            nc.vector.memset(eps_t, EPS)
            ones_t = const.tile([128, 64], bf16)
            nc.vector.memset(ones_t, 1.0)

            # persistent across phases (all bf16 matmul operands).
            # qT/kT hold only the real 64 head dims: score matmuls
            # contract K=64, so no zero-padding rows are needed.
            qkT = persist.tile([64, 5, T], bf16)
            # v with aux columns:
            #  v_aug  [128,TT,65]:  cols 0:64 = v, col 64 = 1  (even head of pair)
            #  v_aug2 [128,TT,128]: col 32 = 1, cols 64:128 = v (odd head of pair)
            v_aug = persist.tile([128, TT, 65], bf16)
            nc.vector.memset(v_aug[:, :, 64:65], 1.0)
            v_aug2 = persist.tile([128, TT, 128], bf16)
            nc.vector.memset(v_aug2[:, :, 0:64], 0.0)
            nc.vector.memset(v_aug2[:, :, 32:33], 1.0)
            ctxB = [persist.tile([128, T], bf16, name=f"ctxB{p}") for p in range(2)]
            dbc = [persist.tile([128, T], f32, name=f"dbc{p}") for p in range(2)]
            wo_sb = [persist.tile([128, D], bf16, name=f"wo{p}") for p in range(2)]
            for p in range(2):
                nc.gpsimd.dma_start(
                    out=wo_sb[p], in_=wo_d[p * 128:(p + 1) * 128, :]
                )
            mpat_sb = persist.tile([128, n_pat, 512], bf16, name="mpat_sb")
            nc.gpsimd.dma_start(
                out=mpat_sb, in_=mpat_d.rearrange("n p f -> p n f")
            )

            # ---------- phase 1: transpose x, project qkv, norm+rope ----------
            with (
                tc.tile_pool(name="p1w", bufs=1) as p1w,
                tc.tile_pool(name="p1s", bufs=3) as p1s,
                tc.tile_pool(name="p1t", bufs=3) as p1t,
                tc.tile_pool(name="ps1a", bufs=3, space="PSUM") as ps1a,
                tc.tile_pool(name="ps1b", bufs=3, space="PSUM") as ps1b,
                tc.tile_pool(name="ps1c", bufs=2, space="PSUM") as ps1c,
            ):
                # weight/table DMAs ride the DVE trigger queue so the
                # per-tt x loads on the sync queue start immediately
                wqkv_sb = p1w.tile([128, CC, NQKV], bf16)
                wqkv_r = wqkv_d.rearrange("(cc p) m -> p cc m", p=128)
                ctab = p1w.tile([128, TT, 5, HD], bf16, name="ctab")
                ctab_r = cosa_d.rearrange("(tt p) (u d) -> p tt u d", p=128, u=5)
                stab = p1w.tile([128, TT, 5, HD], bf16, name="stab")
                stab_r = sina_d.rearrange("(tt p) (u d) -> p tt u d", p=128, u=5)
                # chunked + interleaved so the first tiles' operands land
                # early instead of queueing behind 2.6 MB of tables
                for wc in range(4):
                    sl = slice(wc * 4, (wc + 1) * 4)
                    nc.scalar.dma_start(out=wqkv_sb[:, sl, :],
                                        in_=wqkv_r[:, sl, :])
                    nc.scalar.dma_start(out=ctab[:, sl], in_=ctab_r[:, sl])
                    nc.scalar.dma_start(out=stab[:, sl], in_=stab_r[:, sl])

                for tt in range(TT):
                    x_nat = p1s.tile([128, D], bf16, tag="x_nat")
                    if tt == 0:
                        # split the very first load so the leading
                        # transposes start half a transfer earlier
                        nc.sync.dma_start(out=x_nat[:, 0:1024],
                                          in_=x_d[0:128, 0:1024])
                        nc.sync.dma_start(out=x_nat[:, 1024:2048],
                                          in_=x_d[0:128, 1024:2048])
                    else:
                        nc.sync.dma_start(
                            out=x_nat, in_=x_d[tt * 128:(tt + 1) * 128, :]
                        )
                    xt_col = p1s.tile([128, CC, 128], bf16, tag="xt_col")
                    for cg in range(4):
                        pst = ps1a.tile([128, 512], bf16, tag="pst")
                        for k in range(4):
                            cc = cg * 4 + k
                            nc.tensor.transpose(
                                pst[:, k * 128:(k + 1) * 128],
                                x_nat[:, cc * 128:(cc + 1) * 128],
                                ident,
                            )
                        eng = nc.vector.tensor_copy if cg % 2 == 0 else nc.scalar.copy
                        eng(
                            xt_col[:, cg * 4:(cg + 1) * 4, :]
                            .rearrange("p a b -> p (a b)"),
                            pst,
                        )
                    psqkv = ps1b.tile([128, NQKV], f32, tag="psqkv")
                    for cc in range(CC):
                        mmr(psqkv, xt_col[:, cc, :], wqkv_sb[:, cc, :],
                            start=(cc == 0), stop=(cc == CC - 1))
                    psv = psqkv[:, NQK:NQKV]

                    nc.scalar.copy(v_aug[:, tt, 0:64], psv)
                    nc.scalar.copy(v_aug2[:, tt, 64:128], psv)

                    # rms-norm + rope for 4 q heads + k in one 5-unit batch
                    # (PSUM -> SBUF first: DVE tensor-tensor can't read PSUM)
                    qk5 = p1t.tile([128, 5, HD], f32, tag="qk5")
                    nc.scalar.copy(
                        qk5, psqkv[:, 0:NQK].rearrange("p (u d) -> p u d", u=5)
                    )
                    sq = p1t.tile([128, 5, HD], f32, tag="sq")
                    nc.scalar.activation(
                        sq, psqkv[:, 0:NQK].rearrange("p (u d) -> p u d", u=5),
                        AF.Square,
                    )
                    ssum = p1t.tile([128, 5, 1], f32, tag="ssum")
                    nc.vector.reduce_sum(ssum, sq, axis=AX.X)
                    rinv = p1t.tile([128, 5, 1], f32, tag="rinv")
                    nc.scalar.activation(rinv, ssum, AF.Sqrt,
                                         bias=eps_t[:, 0:1], scale=1.0 / HD)
                    nc.vector.reciprocal(rinv, rinv)
                    qn = p1t.tile([128, 5, HD], bf16, tag="qn")
                    nc.vector.tensor_mul(
                        qn, qk5, rinv.to_broadcast([128, 5, HD])
                    )
                    qr = p1t.tile([128, 5, HD], bf16, tag="qr")
                    nc.vector.tensor_mul(qr, qn, ctab[:, tt, :, :])
                    qrot = p1t.tile([128, 5, HD], bf16, tag="qrot")
                    nc.gpsimd.tensor_mul(
                        qrot[:, :, 0:32], qn[:, :, 32:64],
                        stab[:, tt, :, 0:32],
                    )
                    nc.gpsimd.tensor_mul(
                        qrot[:, :, 32:64], qn[:, :, 0:32],
                        stab[:, tt, :, 32:64],
                    )
                    qb = p1t.tile([128, 5, HD], bf16, tag="qb")
                    nc.vector.tensor_add(qb, qr, qrot)

                    # transpose the 5 units into qT / kT
                    psqt = ps1c.tile([64, 5, 128], bf16, tag="psqt")
                    for u in range(5):
                        nc.tensor.transpose(psqt[:, u, :], qb[:, u, :], ident)
                    nc.vector.tensor_copy(
                        qkT[:, :, tt * 128:(tt + 1) * 128], psqt
                    )

            # ---------- phase 2: attention + denominators + out-proj ----------
            with (
                tc.tile_pool(name="p2e", bufs=8) as p2e,
                tc.tile_pool(name="ps2s", bufs=4, space="PSUM") as ps2s,
                tc.tile_pool(name="ps2c", bufs=2, space="PSUM") as ps2c,
                tc.tile_pool(name="ps2o", bufs=2, space="PSUM") as ps2o,
            ):

                inv_sqrt_d = float(1.0 / np.sqrt(HD))

                def out_proj_quarter(ib, t4):
                    tt = ib * 4 + t4
                    for cb in range(4):
                        pso = ps2o.tile([128, 512], f32, tag="pso")
                        for pair in range(2):
                            mmr(pso,
                                ctxB[pair][:, tt * 128:(tt + 1) * 128],
                                wo_sb[pair][:, cb * 512:(cb + 1) * 512],
                                start=(pair == 0), stop=(pair == 1))
                        ot = p2e.tile([128, 512], bf16, tag="ot")
                        (nc.scalar.copy if ib == IBS - 1
                         else nc.vector.tensor_copy)(ot, pso)
                        nc.sync.dma_start(
                            out=out_d[tt * 128:(tt + 1) * 128,
                                      cb * 512:(cb + 1) * 512],
                            in_=ot,
                        )

                def den_pair(ib, pair, pe, po, den_sb):
                    tail = (ib == IBS - 1 and pair == 1)
                    # denominators: reciprocal (bf16) into SBUF staging at
                    # the same partition rows, then broadcast across the
                    # partition dim with a K=1 ones-matmul (out reuses a
                    # pss ring slot), stage to SBUF, scale ctx into ctxB
                    iw = slice(ib * 512, (ib + 1) * 512)
                    with nc.allow_low_precision(
                        reason="1/den in bf16: 0.4% on softmax scale is "
                               "well inside the 2e-2 tolerance"
                    ):
                        nc.vector.reciprocal(den_sb[32:33, :], po[32:33, :])
                    pdb = ps2s.tile([128, 512], f32, tag="pss")
                    mmr(pdb[0:64, :], ones_t[64:65, :], den_sb[64:65, :],
                        start=True, stop=True)
                    mmr(pdb[64:128, :], ones_t[32:33, :], den_sb[32:33, :],
                        start=True, stop=True)
                    (nc.scalar.copy if tail
                     else nc.vector.tensor_copy)(dbc[pair][:, iw], pdb)
                    # stage ctx PSUM -> SBUF (only DVE/ACT read PSUM),
                    # then scale by 1/den on DVE into bf16
                    ctx_s = p2e.tile([128, 512], f32, tag="ctx_s")
                    nc.vector.tensor_copy(ctx_s[0:64, :], pe[0:64, :])
                    nc.vector.tensor_copy(ctx_s[64:128, :], po[64:128, :])
                    nc.vector.tensor_mul(
                        ctxB[pair][:, iw], ctx_s, dbc[pair][:, iw],
                    )

                for ib in range(IBS):
                    iw = slice(ib * 512, (ib + 1) * 512)
                    jbs = [jb for jb in range(JBS) if status[ib][jb] != SKIP]
                    psc_of = {}
                    den_sb_of = {}
                    for h in range(H_LOC):
                        pair, sub = divmod(h, 2)
                        psc = ps2c.tile([128, 512], f32, tag="psc")
                        psc_of[h] = psc
                        if sub == 0:
                            ctx_out = psc[0:65, :]
                            lhs_of = lambda jb: v_aug[:, jb, :]
                        else:
                            ctx_out = psc
                            lhs_of = lambda jb: v_aug2[:, jb, :]
                        # all QK matmuls first; exp/mask trail on ACT/DVE;
                        # then the PV accumulation chain (never stalls PE)
                        ets = []
                        for jb in jbs:
                            st = status[ib][jb]
                            pre = (pat_prefix[st[1]]
                                   if isinstance(st, tuple) else 0)
                            cw = slice(pre, 512)
                            pss = ps2s.tile([128, 512], f32, tag="pss")
                            mmr(pss[:, cw],
                                qkT[:, 4, jb * 128:(jb + 1) * 128],
                                qkT[:, h, ib * 512 + pre:(ib + 1) * 512],
                                start=True, stop=True)
                            et = p2e.tile([128, 512], bf16, tag="et")
                            nc.scalar.activation(et[:, cw], pss[:, cw],
                                                 AF.Exp, scale=inv_sqrt_d)
                            if isinstance(st, tuple):
                                nc.vector.tensor_mul(
                                    et[:, cw], et[:, cw],
                                    mpat_sb[:, st[1], cw]
                                )
                            ets.append((et, cw))
                        for n, jb in enumerate(jbs):
                            et, cw = ets[n]
                            mmr(ctx_out[:, cw], lhs_of(jb), et[:, cw],
                                start=(n == 0), stop=(n == len(jbs) - 1))
                        # even head: its den-row reciprocal only needs
                        # this head's PSUM - start it before the odd head
                        if sub == 0:
                            den_sb = p2e.tile([65, 512], bf16, tag="den_sb")
                            with nc.allow_low_precision(
                                reason="1/den in bf16, inside 2e-2 tol"
                            ):
                                nc.vector.reciprocal(den_sb[64:65, :],
                                                     psc[64:65, :])
                            den_sb_of[pair] = den_sb
                        else:
                            den_pair(ib, pair, psc_of[h - 1], psc,
                                     den_sb_of.pop(pair))
                            # out-projection lags one i-block behind; its
                            # matmuls fill PE under this block's exp waits
                            if ib > 0:
                                out_proj_quarter(ib - 1, 2 * pair)
                                out_proj_quarter(ib - 1, 2 * pair + 1)
                for t4 in range(4):
                    out_proj_quarter(IBS - 1, t4)

    _split_multiwaits(nc)
    return nc


_CACHE = {}


def _get_program(mask_key, status, n_pat, pat_prefix):
    if mask_key not in _CACHE:
        _CACHE[mask_key] = _build_program(status, n_pat, pat_prefix)
    return _CACHE[mask_key]


def _prepare(x, mask, cos, sin, W_query, W_key, W_value, W_out,
             q_scale, k_scale):
    """Host-side prep: fold scales into rope tables, shard weights,
    classify the mask.  Returns (nc, in_maps)."""
    cos = np.asarray(cos, dtype=np.float32)
    sin = np.asarray(sin, dtype=np.float32)
    W_query = np.asarray(W_query, dtype=np.float32)
    W_key = np.asarray(W_key, dtype=np.float32)
    W_value = np.asarray(W_value, dtype=np.float32)
    W_out = np.asarray(W_out, dtype=np.float32)
    q_scale = np.asarray(q_scale, dtype=np.float32)
    k_scale = np.asarray(k_scale, dtype=np.float32)
    mask = np.asarray(mask)

    xf = np.ascontiguousarray(
        np.asarray(x).reshape(T, D).astype(BF16)
    )

    # rope = qn*cos' + shuffle32(qn)*sin' with the rotate-half signs and the
    # post-norm q/k scales folded into the tables:
    #   rope(s*qn) = qn*(s*cos) + shuffle32(qn)*(shuffle32(s)*sin+-)
    def tables(scale):
        perm = np.concatenate([scale[HD // 2:], scale[:HD // 2]])
        c = (cos * scale[None, :]).astype(np.float32)
        s = (sin * perm[None, :]).astype(np.float32)
        s[:, :HD // 2] *= -1.0
        return c, s

    cq, sq_t = tables(q_scale)
    ck, sk_t = tables(k_scale)
    # 5-unit tables: 4 q heads then k  -> [T, 320]
    cosa = np.ascontiguousarray(
        np.concatenate([cq, cq, cq, cq, ck], axis=1).astype(BF16)
    )
    sina = np.ascontiguousarray(
        np.concatenate([sq_t, sq_t, sq_t, sq_t, sk_t], axis=1).astype(BF16)
    )

    status, patterns, prefixes = _classify_mask(mask)
    nc = _get_program(mask.tobytes(), status, patterns.shape[0], prefixes)
    patterns_bf = patterns.astype(BF16)

    in_maps = []
    for c in range(N_CORES):
        qcols = slice(c * H_LOC * HD, (c + 1) * H_LOC * HD)
        kvcols = slice(c * HD, (c + 1) * HD)
        wqkv = np.concatenate(
            [W_query[:, qcols], W_key[:, kvcols], W_value[:, kvcols]], axis=1
        ).astype(BF16)
        in_maps.append({
            "x": xf,
            "wqkv": np.ascontiguousarray(wqkv),
            "wo": np.ascontiguousarray(W_out[qcols, :].astype(BF16)),
            "cosa": cosa, "sina": sina,
            "mpat": patterns_bf,
        })
    return nc, in_maps


def kernel(x, mask, cos, sin, W_query, W_key, W_value, W_out,
           q_scale, k_scale):
    out_dtype = np.asarray(x).dtype
    nc, in_maps = _prepare(x, mask, cos, sin, W_query, W_key, W_value,
                           W_out, q_scale, k_scale)

    from concourse.bass_utils import run_bass_kernel_spmd

    res = run_bass_kernel_spmd(nc, in_maps, list(range(N_CORES)))
    acc = res.results[0]["out"].astype(np.float32)
    for c in range(1, N_CORES):
        acc = acc + res.results[c]["out"].astype(np.float32)
    return acc.reshape(1, T, D).astype(out_dtype)



# revision 2
# speedup vs baseline: 1.1493x; 1.1493x over previous
"""Grouped-Query Attention kernel v2 for 8 Trainium2 NeuronCores.

Reference model: x[1,2048,2048] -> Q(32 heads x 64) / K,V(8 kv heads x 64),
per-head RMS-norm(Q,K) + RoPE, causal softmax attention, out-projection.

Sharding (tensor-parallel over heads): core c owns Q heads 4c..4c+3 and KV
head c (its GQA group) and W_out rows [256c : 256c+256).  Each core computes
a full-shape partial output; the host sums the 8 partials.

v2 speedups over the 201us baseline:
  - x is transposed on the HOST (no on-chip PE transposes of x)
  - QKV projection runs in fp8 DoubleRow mode (0.5 cyc/col) with 3-term
    residual compensation:  xW ~= x8.W8 + x8.Wr8 + xr8.W8  where
    a = a8 + ar8 splits every operand into fp8 value + fp8 residual.
    The two correction terms share one DoubleRow matmul via its 2 k-groups.
  - PV runs in fp8 DoubleRow over jb-PAIRS: exp outputs e4m3 directly
    (scaled 2^-4; numerator and denominator share the quantized weights so
    softmax stays exactly normalized); v uses fp8 + fp8-residual chains.
  - out-projection is fp8 DoubleRow 3-term (ctx8/cr8 x wo8/wor8).
  - scores stay bf16 (fp8 q/k costs 1.8e-2 of the 2e-2 error budget).
  - diagonal (causally masked) tiles stay bf16 end-to-end: bf16 exp with
    prefix-trimmed windows, 2x-mode DVE pattern multiplies, bf16 PV.
  - weights/x prescaled by G=32 on host to keep fp8 in its normal range;
    compensated exactly via a 1/G ones-column in the den broadcast and a
    final 1/G on the host.
"""

import numpy as np
import ml_dtypes

BF16 = ml_dtypes.bfloat16
F8 = ml_dtypes.float8_e4m3fn

T = 2048
D = 2048
NUM_HEADS = 32
NUM_KV = 8
HD = 64
N_CORES = 8
H_LOC = NUM_HEADS // N_CORES  # 4 q heads per core
EPS = 1e-6
G = 32.0        # fp8 prescale on W_qkv and W_out
ESH = 4         # exp output scaled by 2^-ESH to fit e4m3

TT = T // 128   # 16 t-tiles of 128 rows
CC = D // 128   # 16 contraction chunks
IBS = T // 512  # 4 i-blocks of 512 query positions
JBS = T // 128  # 16 j-blocks of 128 key positions

KEEP = "keep"
SKIP = "skip"
AFFINE = "affine"  # kept for test.py compat; no longer produced


def _classify_mask(mask: np.ndarray):
    """Per (ib, jb) scoresT tile: KEEP / SKIP / ('pat', idx) with deduped
    multiplicative keep-masks in S^T (j, i) layout.  A causal mask yields
    just 4 distinct edge patterns."""
    keep = ~mask
    status = [[KEEP] * JBS for _ in range(IBS)]
    pat_index: dict[bytes, int] = {}
    pats: list[np.ndarray] = []
    for ib in range(IBS):
        for jb in range(JBS):
            sub = keep[ib * 512:(ib + 1) * 512, jb * 128:(jb + 1) * 128]
            if sub.all():
                status[ib][jb] = KEEP
            elif not sub.any():
                status[ib][jb] = SKIP
            else:
                key = sub.tobytes()
                if key not in pat_index:
                    pat_index[key] = len(pats)
                    pats.append(sub.T.astype(np.float32))  # [128 j, 512 i]
                status[ib][jb] = ("pat", pat_index[key])
    patterns = (
        np.stack(pats) if pats else np.zeros((1, 128, 512), dtype=np.float32)
    )
    # leading i-columns that are fully masked in each pattern: the score
    # matmul / exp / PV only need the live suffix
    prefixes = []
    for p in patterns:
        alive = p.any(axis=0)
        prefixes.append(int(alive.argmax()) if alive.any() else 512)
    return status, patterns, prefixes


def _split_multiwaits(nc):
    """walrus in this container accepts only ONE sync-wait per instruction;
    hoist extra waits onto preceding same-engine NoOps (program order on the
    engine queue preserves the gating)."""
    import bass_rust
    from concourse import mybir

    n_fixed = 0
    for fn in nc.m.functions:
        for bb in fn.blocks:
            out = []
            for ins in bb.instructions:
                si = ins.sync_info
                if si is not None and si.on_wait and len(si.on_wait) > 1:
                    waits = list(si.on_wait)
                    ups = list(si.on_update) if si.on_update else []
                    for k, w in enumerate(waits[:-1]):
                        nop = mybir.InstNoOp(
                            name=f"{ins.name}-wnop{k}", ins=[], outs=[]
                        )
                        nop.engine = ins.engine
                        nop.sync_info = bass_rust.SyncInfo(
                            on_wait=[w], on_update=[]
                        )
                        out.append(nop)
                    ins.sync_info = bass_rust.SyncInfo(
                        on_wait=[waits[-1]], on_update=ups
                    )
                    n_fixed += 1
                out.append(ins)
            bb.instructions = out
    return n_fixed


def _plan_jbs(status, ib):
    """Split live jbs of an i-block into DR pairs (full tiles) and a bf16
    diag list [(jb, prefix)]."""
    full = [jb for jb in range(JBS) if status[ib][jb] == KEEP]
    diag = [(jb, st[1]) for jb in range(JBS)
            if isinstance(st := status[ib][jb], tuple)]
    if len(full) % 2:  # defensive: odd full count -> route one via diag path
        diag.append((full.pop(), None))
    pairs = [(full[2 * p], full[2 * p + 1]) for p in range(len(full) // 2)]
    return pairs, diag


def _build_program(status, n_pat, pat_prefix):
    import concourse.bass as bass
    import concourse.mybir as mybir
    import concourse.tile as tile
    from concourse.masks import make_identity

    f32 = mybir.dt.float32
    bf16 = mybir.dt.bfloat16
    f8 = mybir.dt.float8e4
    AX = mybir.AxisListType
    AF = mybir.ActivationFunctionType
    DR = mybir.MatmulPerfMode.DoubleRow

    nc = bass.Bass("TRN2", num_devices=N_CORES)
    # x: [128, TT, CC, 2, 128]  slot0 = xr8, slot1 = x8 (fp8, G-free)
    xc_d = nc.declare_dram_parameter("xc", [128, TT, CC, 2, 128], f8,
                                     isOutput=False)
    # W_qkv: [128, CC, 2, 384]  slot0 = W8, slot1 = Wr8 (fp8, xG)
    wc_d = nc.declare_dram_parameter("wc", [128, CC, 2, 384], f8,
                                     isOutput=False)
    # W_out: [128, 2(pair), 2(slot), D]  slot0 = wor8, slot1 = wo8 (fp8, xG)
    woc_d = nc.declare_dram_parameter("woc", [128, 2, 2, D], f8,
                                      isOutput=False)
    # rope tables, 2 units (q, k), scales folded in
    cosa_d = nc.declare_dram_parameter("cosa", [T, 2 * HD], bf16,
                                       isOutput=False)
    sina_d = nc.declare_dram_parameter("sina", [T, 2 * HD], bf16,
                                       isOutput=False)
    mpat_d = nc.declare_dram_parameter(
        "mpat", [n_pat, 128, 512], bf16, isOutput=False
    )
    out_d = nc.declare_dram_parameter("out", [T, D], bf16, isOutput=True)

    NQKV = H_LOC * HD + 2 * HD  # 384: q heads, then k, then v
    NQK = (H_LOC + 1) * HD      # 320: q heads + k (norm/rope batch)

    mmr = nc.tensor.matmul
    ib_pairs_diag = [_plan_jbs(status, ib) for ib in range(IBS)]

    with tile.TileContext(nc) as tc:
        with (
            tc.tile_pool(name="const", bufs=1) as const,
            tc.tile_pool(name="persist", bufs=1) as persist,
        ):
            ident = const.tile([128, 128], bf16)
            eps_t = const.tile([128, 1], f32)
            ebias_t = const.tile([128, 1], f32)
            g_t = const.tile([128, 64], bf16)

            qkT = persist.tile([64, 5, T], bf16)
            # staged qkv projection (f32), persistent so v-side copies can
            # batch over 4 t-tiles at a time
            qkv_sp = persist.tile([128, TT, NQKV], f32, name="qkv_sp")
            # v variants (a: even head, 68 cols, den col 64;
            #             b: odd head, 128 cols, den col 32, v at 64:128)
            v8a = persist.tile([128, TT, 128], f8, name="v8a")
            v8b = persist.tile([128, TT, 128], f8, name="v8b")
            vr8a = persist.tile([128, TT, 128], f8, name="vr8a")
            vr8b = persist.tile([128, TT, 128], f8, name="vr8b")
            vba = persist.tile([128, TT, 68], bf16, name="vba")
            vbb = persist.tile([128, TT, 128], bf16, name="vbb")
            # ctx in fp8 + residual: [128, pair, slot(ctx8,cr8), T]
            ctxc8 = persist.tile([128, 2, 2, T], f8, name="ctxc8")
            dbc = [persist.tile([128, T], bf16, name=f"dbc{p}")
                   for p in range(2)]
            woc_sb = persist.tile([128, 2, 2, D], f8, name="woc_sb")
            mpat_sb = persist.tile([128, n_pat, 512], bf16, name="mpat_sb")

            # ---------- phase 1: project qkv (fp8 3-term DR), norm+rope ----
            with (
                tc.tile_pool(name="p1w", bufs=1) as p1w,
                tc.tile_pool(name="p1x", bufs=5) as p1x,
                tc.tile_pool(name="p1t", bufs=5) as p1t,
                tc.tile_pool(name="ps1b", bufs=3, space="PSUM") as ps1b,
                tc.tile_pool(name="ps1c", bufs=2, space="PSUM") as ps1c,
            ):
                wc_sb = p1w.tile([128, CC, 2, NQKV], f8)
                ctab = p1w.tile([128, TT, 2, HD], bf16, name="ctab")
                ctab_r = cosa_d.rearrange("(tt p) (u d) -> p tt u d",
                                          p=128, u=2)
                stab = p1w.tile([128, TT, 2, HD], bf16, name="stab")
                stab_r = sina_d.rearrange("(tt p) (u d) -> p tt u d",
                                          p=128, u=2)
                # x/qkv-weight/table loads first; phase-2-only tensors after
                for wq in range(4):
                    sl = slice(wq * 4, (wq + 1) * 4)
                    nc.scalar.dma_start(out=wc_sb[:, sl], in_=wc_d[:, sl])
                nc.scalar.dma_start(out=ctab, in_=ctab_r)
                nc.scalar.dma_start(out=stab, in_=stab_r)
                # constants / aux columns after the DMA triggers
                make_identity(nc, ident)
                nc.vector.memset(eps_t, EPS * G * G)
                nc.vector.memset(ebias_t, -ESH * float(np.log(2.0)))
                nc.vector.memset(g_t, 1.0 / G)
                nc.gpsimd.memset(v8a[:, :, 64:128], 0.0)
                nc.gpsimd.memset(v8a[:, :, 64:65], 1.0)
                nc.vector.memset(v8b[:, :, 0:64], 0.0)
                nc.vector.memset(v8b[:, :, 32:33], 1.0)
                nc.gpsimd.memset(vr8a[:, :, 64:128], 0.0)
                nc.vector.memset(vr8b[:, :, 0:64], 0.0)
                nc.gpsimd.memset(vba[:, :, 64:68], 0.0)
                nc.gpsimd.memset(vba[:, :, 64:65], 1.0)
                nc.vector.memset(vbb[:, :, 0:64], 0.0)
                nc.vector.memset(vbb[:, :, 32:33], 1.0)

                pending_qb = []

                def flush_qb():
                    for qb_p, tt_p in pending_qb:
                        psqt = ps1c.tile([64, 5, 128], bf16, tag="psqt")
                        for u in range(5):
                            nc.tensor.transpose(psqt[:, u, :], qb_p[:, u, :],
                                                ident)
                        nc.scalar.copy(
                            qkT[:, :, tt_p * 128:(tt_p + 1) * 128], psqt
                        )
                    pending_qb.clear()

                def emit_v_batch(g):
                    # batched fp8/bf16 v staging for t-tiles 4g..4g+3
                    ts4 = slice(4 * g, 4 * g + 4)
                    vf = qkv_sp[:, ts4, NQK:NQKV]
                    nc.gpsimd.tensor_copy(v8a[:, ts4, 0:64], vf)
                    nc.gpsimd.tensor_copy(v8b[:, ts4, 64:128], vf)
                    nc.gpsimd.tensor_sub(vr8a[:, ts4, 0:64], vf,
                                         v8a[:, ts4, 0:64])
                    nc.gpsimd.tensor_copy(vr8b[:, ts4, 64:128],
                                          vr8a[:, ts4, 0:64])
                    nc.gpsimd.tensor_copy(vba[:, ts4, 0:64], vf)
                    nc.gpsimd.tensor_copy(vbb[:, ts4, 64:128], vf)

                for tt in range(TT):
                    xcr = p1x.tile([128, CC, 2, 128], f8, tag="xcr")
                    nc.sync.dma_start(out=xcr, in_=xc_d[:, tt])
                    psqkv = ps1b.tile([128, NQKV], f32, tag="psqkv")
                    # transposes of the PREVIOUS tt go first so the PE
                    # never waits on the rope chain
                    flush_qb()
                    # main: x8 (slot1) x W8 (slot0), cc-pairs as DR groups
                    for c2 in range(CC // 2):
                        mmr(psqkv, xcr[:, 2 * c2:2 * c2 + 2, 1, :],
                            wc_sb[:, 2 * c2:2 * c2 + 2, 0, :],
                            start=(c2 == 0), stop=False, perf_mode=DR)
                    # corr: (xr8, x8) x (W8, Wr8) = xr8.W8 + x8.Wr8
                    for cc in range(CC):
                        mmr(psqkv, xcr[:, cc, :, :], wc_sb[:, cc, :, :],
                            start=False, stop=(cc == CC - 1), perf_mode=DR)

                    # single fast staging copy frees the psum ring quickly
                    nc.vector.tensor_copy(qkv_sp[:, tt], psqkv)
                    qk5 = qkv_sp[:, tt, 0:NQK].rearrange(
                        "p (u d) -> p u d", u=5)

                    # rope FIRST on the raw (G-scaled) projections, the
                    # rms-norm scalar lands at the end: rope commutes with
                    # the per-(token,unit) rinv, so the sqrt chain computes
                    # concurrently instead of gating the whole chain
                    sq = p1t.tile([128, 5, HD], f32, tag="sq")
                    nc.scalar.activation(sq, qk5, AF.Square)
                    ssum = p1t.tile([128, 5, 1], f32, tag="ssum")
                    nc.vector.reduce_sum(ssum, sq, axis=AX.X)
                    rinv = p1t.tile([128, 5, 1], f32, tag="rinv")
                    nc.scalar.activation(rinv, ssum, AF.Sqrt,
                                         bias=eps_t[:, 0:1], scale=1.0 / HD)
                    nc.vector.reciprocal(rinv, rinv)
                    cq = ctab[:, tt, 0:1, :].to_broadcast([128, 4, HD])
                    qr = p1t.tile([128, 5, HD], bf16, tag="qr")
                    nc.vector.tensor_mul(qr[:, 0:4, :], qk5[:, 0:4, :], cq)
                    nc.vector.tensor_mul(qr[:, 4:5, :], qk5[:, 4:5, :],
                                         ctab[:, tt, 1:2, :])
                    qrot = p1t.tile([128, 5, HD], bf16, tag="qrot")
                    nc.gpsimd.tensor_mul(
                        qrot[:, 0:4, 0:32], qk5[:, 0:4, 32:64],
                        stab[:, tt, 0:1, 0:32].to_broadcast([128, 4, 32]),
                    )
                    nc.gpsimd.tensor_mul(
                        qrot[:, 0:4, 32:64], qk5[:, 0:4, 0:32],
                        stab[:, tt, 0:1, 32:64].to_broadcast([128, 4, 32]),
                    )
                    nc.gpsimd.tensor_mul(
                        qrot[:, 4:5, 0:32], qk5[:, 4:5, 32:64],
                        stab[:, tt, 1:2, 0:32],
                    )
                    nc.gpsimd.tensor_mul(
                        qrot[:, 4:5, 32:64], qk5[:, 4:5, 0:32],
                        stab[:, tt, 1:2, 32:64],
                    )
                    qa = p1t.tile([128, 5, HD], bf16, tag="qa")
                    nc.vector.tensor_add(qa, qr, qrot)
                    qb = p1t.tile([128, 5, HD], bf16, tag="qb")
                    nc.vector.tensor_mul(qb, qa,
                                         rinv.to_broadcast([128, 5, HD]))
                    pending_qb.append((qb, tt))
                    if tt % 4 == 3:
                        emit_v_batch(tt // 4)
                    if tt == 10:
                        # phase-2-only tensors ride in after the x stream
                        # has mostly landed (the 360GB/s DMA roof gates
                        # phase 1)
                        nc.scalar.dma_start(out=woc_sb, in_=woc_d[:])
                        nc.scalar.dma_start(
                            out=mpat_sb, in_=mpat_d.rearrange("n p f -> p n f")
                        )
                flush_qb()

            # ---------- phase 2: attention + den + out-proj ----------
            with (
                tc.tile_pool(name="p2e8", bufs=14) as p2e8,
                tc.tile_pool(name="p2eb", bufs=7) as p2eb,
                tc.tile_pool(name="p2e", bufs=4) as p2e,
                tc.tile_pool(name="p2o", bufs=2) as p2o,
                tc.tile_pool(name="ps2s", bufs=3, space="PSUM") as ps2s,
                tc.tile_pool(name="ps2c", bufs=2, space="PSUM") as ps2c,
            ):
                inv_sqrt_d = float(1.0 / np.sqrt(HD))
                ot_n = [0]
                act_free = [False]

                def out_proj_quarter(ib, t4, tail=False):
                    tail = tail or act_free[0]
                    # 3-term fp8 DR out-proj; psum slots come from the shared
                    # score ring, staging copies alternate DVE / ACT
                    tt = ib * 4 + t4
                    tw = slice(tt * 128, (tt + 1) * 128)
                    ot = p2o.tile([128, D], bf16, tag="ot")
                    if tail:
                        # the score ring is free once the last exps are in
                        # flight: 2 cb per 2-bank slot, wide staging copies
                        for half4 in range(2):
                            pso = ps2s.tile([128, 2, 512], f32, tag="pss")
                            for hcb in range(2):
                                cb = half4 * 2 + hcb
                                cw = slice(cb * 512, (cb + 1) * 512)
                                mmr(pso[:, hcb, :], ctxc8[:, :, 0, tw],
                                    woc_sb[:, :, 1, cw],
                                    start=True, stop=False, perf_mode=DR)
                                for p in range(2):
                                    mmr(pso[:, hcb, :], ctxc8[:, p, :, tw],
                                        woc_sb[:, p, :, cw],
                                        start=False, stop=(p == 1),
                                        perf_mode=DR)
                            eng = (nc.scalar.copy if ot_n[0] % 2 == 1
                                   else nc.vector.tensor_copy)
                            ot_n[0] += 1
                            eng(ot[:, half4 * 1024:(half4 + 1) * 1024]
                                .rearrange("p (a b) -> p a b", a=2), pso)
                    else:
                        for cb in range(4):
                            cw = slice(cb * 512, (cb + 1) * 512)
                            pso = ps2c.tile([128, 512], f32, tag="psc")
                            mmr(pso, ctxc8[:, :, 0, tw], woc_sb[:, :, 1, cw],
                                start=True, stop=False, perf_mode=DR)
                            for p in range(2):
                                mmr(pso, ctxc8[:, p, :, tw],
                                    woc_sb[:, p, :, cw],
                                    start=False, stop=(p == 1), perf_mode=DR)
                            eng = (nc.scalar.copy
                                   if (act_free[0] and ot_n[0] % 2 == 1)
                                   else nc.vector.tensor_copy)
                            ot_n[0] += 1
                            eng(ot[:, cw], pso)
                    nc.sync.dma_start(out=out_d[tt * 128:(tt + 1) * 128, :],
                                      in_=ot)

                def den_pair(ib, pair, pe, po, den_sb):
                    # reciprocal of both heads' denominators -> broadcast
                    # across partitions with K=1 matmuls (value 1/G folds
                    # away the v-side G prescale) -> normalize ctx straight
                    # from psum, emit fp8 ctx + fp8 residual for out-proj.
                    iw = slice(ib * 512, (ib + 1) * 512)
                    with nc.allow_low_precision(
                        reason="1/den in bf16: 0.4% on softmax scale is "
                               "well inside the 2e-2 tolerance"
                    ):
                        nc.vector.reciprocal(den_sb[32:33, :], po[32:33, :])
                    pdb = ps2s.tile([128, 2, 512], f32, tag="pss")
                    mmr(pdb[0:64, 0, :], g_t[64:65, :], den_sb[64:65, :],
                        start=True, stop=True)
                    mmr(pdb[64:128, 0, :], g_t[32:33, :], den_sb[32:33, :],
                        start=True, stop=True)
                    nc.vector.tensor_copy(dbc[pair][:, iw], pdb[:, 0, :])
                    ctx_n = p2e.tile([128, 512], bf16, tag="ctx_n")
                    nc.vector.tensor_mul(ctx_n[0:64, :], pe[0:64, :],
                                         dbc[pair][0:64, iw])
                    nc.vector.tensor_mul(ctx_n[64:128, :], po[64:128, :],
                                         dbc[pair][64:128, iw])
                    nc.gpsimd.tensor_copy(ctxc8[:, pair, 0, iw], ctx_n)
                    nc.gpsimd.tensor_sub(ctxc8[:, pair, 1, iw], ctx_n,
                                         ctxc8[:, pair, 0, iw])

                def emit_scores(h, ib):
                    iw = slice(ib * 512, (ib + 1) * 512)
                    pairs, diag = ib_pairs_diag[ib]
                    et8s = []
                    for (j0, j1) in pairs:
                        pss = ps2s.tile([128, 2, 512], f32, tag="pss")
                        for half, jb in enumerate((j0, j1)):
                            mmr(pss[:, half, :],
                                qkT[:, 4, jb * 128:(jb + 1) * 128],
                                qkT[:, h, iw],
                                start=True, stop=True)
                        et8 = p2e8.tile([128, 2, 512], f8, tag="et8")
                        nc.scalar.activation(et8, pss, AF.Exp,
                                             scale=inv_sqrt_d,
                                             bias=ebias_t[:, 0:1])
                        et8s.append(et8)
                    etbs = []
                    for n in range(0, len(diag), 2):
                        dgrp = diag[n:n + 2]
                        pss = ps2s.tile([128, 2, 512], f32, tag="pss")
                        pre_g = 512
                        for half, (jb, pat) in enumerate(dgrp):
                            pre = pat_prefix[pat] if pat is not None else 0
                            pre_g = min(pre_g, pre)
                            mmr(pss[:, half, pre:512],
                                qkT[:, 4, jb * 128:(jb + 1) * 128],
                                qkT[:, h, ib * 512 + pre:(ib + 1) * 512],
                                start=True, stop=True)
                        etb = p2eb.tile([128, 2, 512], bf16, tag="etb")
                        nc.scalar.activation(etb[:, :, pre_g:512],
                                             pss[:, :, pre_g:512],
                                             AF.Exp, scale=inv_sqrt_d,
                                             bias=ebias_t[:, 0:1])
                        meng = (nc.vector.tensor_mul if ib == 0
                                else nc.gpsimd.tensor_mul)
                        for half, (jb, pat) in enumerate(dgrp):
                            if pat is None:
                                continue
                            pre = pat_prefix[pat]
                            meng(
                                etb[:, half, pre:512],
                                etb[:, half, pre:512],
                                mpat_sb[:, pat, pre:512],
                            )
                        etbs.append((etb, dgrp))
                    return et8s, etbs

                psc_of = {}
                den_sb_of = {}
                wo_queue = []

                def emit_pv(h, ib, et8s, etbs):
                    pairs, diag = ib_pairs_diag[ib]
                    pair, sub = divmod(h, 2)
                    psc = ps2c.tile([128, 512], f32, tag="psc")
                    psc_of[h, ib] = psc
                    if sub == 0:
                        ctx_out = psc
                        ctx_bout = psc[0:68, :]
                        va8, vr8, vab = v8a, vr8a, vba
                    else:
                        ctx_out = psc
                        ctx_bout = psc
                        va8, vr8, vab = v8b, vr8b, vbb
                    n_mm = 2 * len(pairs) + len(diag)
                    k = 0
                    for n, (j0, j1) in enumerate(pairs):
                        b2 = j0 // 2
                        assert j1 == j0 + 1 and j0 % 2 == 0
                        for vv in (va8, vr8):
                            mmr(ctx_out, vv[:, 2 * b2:2 * b2 + 2, :],
                                et8s[n], start=(k == 0),
                                stop=(k == n_mm - 1), perf_mode=DR)
                            k += 1
                    for etb, dgrp in etbs:
                        for half, (jb, pat) in enumerate(dgrp):
                            pre = (pat_prefix[pat]
                                   if pat is not None else 0)
                            mmr(ctx_bout[:, pre:512], vab[:, jb, :],
                                etb[:, half, pre:512],
                                start=(k == 0), stop=(k == n_mm - 1))
                            k += 1
                    if sub == 0:
                        den_sb = p2e.tile([65, 512], bf16, tag="den_sb")
                        with nc.allow_low_precision(
                            reason="1/den in bf16, inside 2e-2 tol"
                        ):
                            nc.vector.reciprocal(den_sb[64:65, :],
                                                 psc[64:65, :])
                        den_sb_of[pair, ib] = den_sb
                    else:
                        den_pair(ib, pair, psc_of[h - 1, ib], psc,
                                 den_sb_of.pop((pair, ib)))
                        for _ in range(2):
                            if wo_queue:
                                out_proj_quarter(*wo_queue.pop(0))

                # software pipeline: scores/exps of unit n overlap the PV /
                # den / out-proj of unit n-1 so the in-order PE queue never
                # parks on an exp wait.  Unit order: ib0 zipped with ib3
                # (tiny + huge complement each other and cover the phase-1
                # seam), then ib2, then ib1.  Completed i-blocks enqueue
                # out-proj quarters, popped two per den event.
                # pair-granular order: even/odd heads of a pair stay
                # adjacent (the 2-slot psc ring frees at each den event)
                units = [(h, ib) for ib in (1, 3, 2, 0)
                         for h in range(H_LOC)]
                done_cnt = {}
                prev = None
                for n, (h, ib) in enumerate(units):
                    act_free[0] = n >= 12
                    ets = emit_scores(h, ib)
                    if prev is not None:
                        emit_pv(prev[0], prev[1], *prev[2])
                        pib = prev[1]
                        done_cnt[pib] = done_cnt.get(pib, 0) + 1
                        if done_cnt[pib] == H_LOC:
                            wo_queue.extend((pib, t4) for t4 in range(4))
                    prev = (h, ib, ets)
                act_free[0] = True
                emit_pv(prev[0], prev[1], *prev[2])
                wo_queue.extend((prev[1], t4) for t4 in range(4))
                while wo_queue:
                    out_proj_quarter(*wo_queue.pop(0), tail=True)

    _split_multiwaits(nc)
    return nc


_CACHE = {}


def _get_program(mask_key, status, n_pat, pat_prefix):
    if mask_key not in _CACHE:
        _CACHE[mask_key] = _build_program(status, n_pat, pat_prefix)
    return _CACHE[mask_key]


def _f8_pair(a):
    """Split a into (fp8 value, fp8 residual)."""
    hi = a.astype(F8)
    lo = (a - hi.astype(np.float32)).astype(F8)
    return hi, lo


def _prepare(x, mask, cos, sin, W_query, W_key, W_value, W_out,
             q_scale, k_scale):
    """Host-side prep: transpose+fp8-split x, fold scales into rope tables,
    shard + fp8-split weights, classify the mask."""
    cos = np.asarray(cos, dtype=np.float32)
    sin = np.asarray(sin, dtype=np.float32)
    W_query = np.asarray(W_query, dtype=np.float32)
    W_key = np.asarray(W_key, dtype=np.float32)
    W_value = np.asarray(W_value, dtype=np.float32)
    W_out = np.asarray(W_out, dtype=np.float32)
    q_scale = np.asarray(q_scale, dtype=np.float32)
    k_scale = np.asarray(k_scale, dtype=np.float32)
    mask = np.asarray(mask)

    xT = np.asarray(x, dtype=np.float32).reshape(T, D).T  # [D, T]
    x8, xr8 = _f8_pair(xT)
    # xc [128, TT, CC, 2, 128]: slot0 = xr8, slot1 = x8
    xv = np.stack([xr8, x8], axis=0).reshape(2, CC, 128, TT, 128)
    xc = np.ascontiguousarray(xv.transpose(2, 3, 1, 0, 4))

    # rope = qn*cos' + shuffle32(qn)*sin' with the rotate-half signs and the
    # post-norm q/k scales folded into the tables
    def tables(scale):
        perm = np.concatenate([scale[HD // 2:], scale[:HD // 2]])
        c = (cos * scale[None, :]).astype(np.float32)
        s = (sin * perm[None, :]).astype(np.float32)
        s[:, :HD // 2] *= -1.0
        return c, s

    cq, sq_t = tables(q_scale)
    ck, sk_t = tables(k_scale)
    cosa = np.ascontiguousarray(
        np.concatenate([cq, ck], axis=1).astype(BF16)
    )
    sina = np.ascontiguousarray(
        np.concatenate([sq_t, sk_t], axis=1).astype(BF16)
    )

    status, patterns, prefixes = _classify_mask(mask)
    nc = _get_program(mask.tobytes(), status, patterns.shape[0], prefixes)
    patterns_bf = patterns.astype(BF16)

    in_maps = []
    for c in range(N_CORES):
        qcols = slice(c * H_LOC * HD, (c + 1) * H_LOC * HD)
        kvcols = slice(c * HD, (c + 1) * HD)
        wqkv = np.concatenate(
            [W_query[:, qcols], W_key[:, kvcols], W_value[:, kvcols]], axis=1
        ) * G
        w8, wr8 = _f8_pair(wqkv)  # [D, 384]
        # wc [128, CC, 2, 384]: slot0 = W8, slot1 = Wr8
        wc = np.ascontiguousarray(
            np.stack([w8, wr8], axis=0).reshape(2, CC, 128, NQKV_HOST)
            .transpose(2, 1, 0, 3)
        )
        woG = W_out[qcols, :] * G  # [256, D]
        wo8, wor8 = _f8_pair(woG)
        # woc [128, 2(pair), 2(slot), D]: slot0 = wor8, slot1 = wo8
        woc = np.ascontiguousarray(
            np.stack([wor8, wo8], axis=0).reshape(2, 2, 128, D)
            .transpose(2, 1, 0, 3)
        )
        in_maps.append({
            "xc": xc,
            "wc": wc,
            "woc": woc,
            "cosa": cosa, "sina": sina,
            "mpat": patterns_bf,
        })
    return nc, in_maps


NQKV_HOST = H_LOC * HD + 2 * HD


def kernel(x, mask, cos, sin, W_query, W_key, W_value, W_out,
           q_scale, k_scale):
    out_dtype = np.asarray(x).dtype
    nc, in_maps = _prepare(x, mask, cos, sin, W_query, W_key, W_value,
                           W_out, q_scale, k_scale)

    from concourse.bass_utils import run_bass_kernel_spmd

    res = run_bass_kernel_spmd(nc, in_maps, list(range(N_CORES)))
    acc = res.results[0]["out"].astype(np.float32)
    for c in range(1, N_CORES):
        acc = acc + res.results[c]["out"].astype(np.float32)
    acc *= 1.0 / G  # compensate the W_out prescale
    return acc.reshape(1, T, D).astype(out_dtype)


# revision 3
# speedup vs baseline: 1.1635x; 1.0123x over previous
"""Grouped-Query Attention kernel v2 for 8 Trainium2 NeuronCores.

Reference model: x[1,2048,2048] -> Q(32 heads x 64) / K,V(8 kv heads x 64),
per-head RMS-norm(Q,K) + RoPE, causal softmax attention, out-projection.

Sharding (tensor-parallel over heads): core c owns Q heads 4c..4c+3 and KV
head c (its GQA group) and W_out rows [256c : 256c+256).  Each core computes
a full-shape partial output; the host sums the 8 partials.

v2 speedups over the 201us baseline:
  - x is transposed on the HOST (no on-chip PE transposes of x)
  - QKV projection runs in fp8 DoubleRow mode (0.5 cyc/col) with 3-term
    residual compensation:  xW ~= x8.W8 + x8.Wr8 + xr8.W8  where
    a = a8 + ar8 splits every operand into fp8 value + fp8 residual.
    The two correction terms share one DoubleRow matmul via its 2 k-groups.
  - PV runs in fp8 DoubleRow over jb-PAIRS: exp outputs e4m3 directly
    (scaled 2^-4; numerator and denominator share the quantized weights so
    softmax stays exactly normalized); v uses fp8 + fp8-residual chains.
  - out-projection is fp8 DoubleRow 3-term (ctx8/cr8 x wo8/wor8).
  - scores stay bf16 (fp8 q/k costs 1.8e-2 of the 2e-2 error budget).
  - diagonal (causally masked) tiles stay bf16 end-to-end: bf16 exp with
    prefix-trimmed windows, 2x-mode DVE pattern multiplies, bf16 PV.
  - weights/x prescaled by G=32 on host to keep fp8 in its normal range;
    compensated exactly via a 1/G ones-column in the den broadcast and a
    final 1/G on the host.
"""

import numpy as np
import ml_dtypes

BF16 = ml_dtypes.bfloat16
F8 = ml_dtypes.float8_e4m3fn

T = 2048
D = 2048
NUM_HEADS = 32
NUM_KV = 8
HD = 64
N_CORES = 8
H_LOC = NUM_HEADS // N_CORES  # 4 q heads per core
EPS = 1e-6
G = 32.0        # fp8 prescale on W_qkv and W_out
ESH = 4         # exp output scaled by 2^-ESH to fit e4m3

TT = T // 128   # 16 t-tiles of 128 rows
CC = D // 128   # 16 contraction chunks
IBS = T // 512  # 4 i-blocks of 512 query positions
JBS = T // 128  # 16 j-blocks of 128 key positions

KEEP = "keep"
SKIP = "skip"
AFFINE = "affine"  # kept for test.py compat; no longer produced


def _classify_mask(mask: np.ndarray):
    """Per (ib, jb) scoresT tile: KEEP / SKIP / ('pat', idx) with deduped
    multiplicative keep-masks in S^T (j, i) layout.  A causal mask yields
    just 4 distinct edge patterns."""
    keep = ~mask
    status = [[KEEP] * JBS for _ in range(IBS)]
    pat_index: dict[bytes, int] = {}
    pats: list[np.ndarray] = []
    for ib in range(IBS):
        for jb in range(JBS):
            sub = keep[ib * 512:(ib + 1) * 512, jb * 128:(jb + 1) * 128]
            if sub.all():
                status[ib][jb] = KEEP
            elif not sub.any():
                status[ib][jb] = SKIP
            else:
                key = sub.tobytes()
                if key not in pat_index:
                    pat_index[key] = len(pats)
                    pats.append(sub.T.astype(np.float32))  # [128 j, 512 i]
                status[ib][jb] = ("pat", pat_index[key])
    patterns = (
        np.stack(pats) if pats else np.zeros((1, 128, 512), dtype=np.float32)
    )
    # leading i-columns that are fully masked in each pattern: the score
    # matmul / exp / PV only need the live suffix
    prefixes = []
    for p in patterns:
        alive = p.any(axis=0)
        prefixes.append(int(alive.argmax()) if alive.any() else 512)
    return status, patterns, prefixes


def _split_multiwaits(nc):
    """walrus in this container accepts only ONE sync-wait per instruction;
    hoist extra waits onto preceding same-engine NoOps (program order on the
    engine queue preserves the gating)."""
    import bass_rust
    from concourse import mybir

    n_fixed = 0
    for fn in nc.m.functions:
        for bb in fn.blocks:
            out = []
            for ins in bb.instructions:
                si = ins.sync_info
                if si is not None and si.on_wait and len(si.on_wait) > 1:
                    waits = list(si.on_wait)
                    ups = list(si.on_update) if si.on_update else []
                    for k, w in enumerate(waits[:-1]):
                        nop = mybir.InstNoOp(
                            name=f"{ins.name}-wnop{k}", ins=[], outs=[]
                        )
                        nop.engine = ins.engine
                        nop.sync_info = bass_rust.SyncInfo(
                            on_wait=[w], on_update=[]
                        )
                        out.append(nop)
                    ins.sync_info = bass_rust.SyncInfo(
                        on_wait=[waits[-1]], on_update=ups
                    )
                    n_fixed += 1
                out.append(ins)
            bb.instructions = out
    return n_fixed


def _plan_jbs(status, ib):
    """Split live jbs of an i-block into DR pairs (full tiles) and a bf16
    diag list [(jb, prefix)]."""
    full = [jb for jb in range(JBS) if status[ib][jb] == KEEP]
    diag = [(jb, st[1]) for jb in range(JBS)
            if isinstance(st := status[ib][jb], tuple)]
    if len(full) % 2:  # defensive: odd full count -> route one via diag path
        diag.append((full.pop(), None))
    pairs = [(full[2 * p], full[2 * p + 1]) for p in range(len(full) // 2)]
    return pairs, diag


def _build_program(status, n_pat, pat_prefix):
    import concourse.bass as bass
    import concourse.mybir as mybir
    import concourse.tile as tile
    from concourse.masks import make_identity

    f32 = mybir.dt.float32
    bf16 = mybir.dt.bfloat16
    f8 = mybir.dt.float8e4
    AX = mybir.AxisListType
    AF = mybir.ActivationFunctionType
    DR = mybir.MatmulPerfMode.DoubleRow

    nc = bass.Bass("TRN2", num_devices=N_CORES)
    # x: [128, TT, CC, 2, 128]  slot0 = xr8, slot1 = x8 (fp8, G-free)
    xc_d = nc.declare_dram_parameter("xc", [128, TT, CC, 2, 128], f8,
                                     isOutput=False)
    # W_qkv: [128, CC, 2, 384]  slot0 = W8, slot1 = Wr8 (fp8, xG)
    wc_d = nc.declare_dram_parameter("wc", [128, CC, 2, 384], f8,
                                     isOutput=False)
    # W_out: [128, 2(pair), 2(slot), D]  slot0 = wor8, slot1 = wo8 (fp8, xG)
    woc_d = nc.declare_dram_parameter("woc", [128, 2, 2, D], f8,
                                      isOutput=False)
    # rope tables, 2 units (q, k), scales folded in
    cosa_d = nc.declare_dram_parameter("cosa", [T, 2 * HD], bf16,
                                       isOutput=False)
    sina_d = nc.declare_dram_parameter("sina", [T, 2 * HD], bf16,
                                       isOutput=False)
    mpat_d = nc.declare_dram_parameter(
        "mpat", [n_pat, 128, 512], bf16, isOutput=False
    )
    out_d = nc.declare_dram_parameter("out", [T, D], bf16, isOutput=True)

    NQKV = H_LOC * HD + 2 * HD  # 384: q heads, then k, then v
    NQK = (H_LOC + 1) * HD      # 320: q heads + k (norm/rope batch)

    mmr = nc.tensor.matmul
    ib_pairs_diag = [_plan_jbs(status, ib) for ib in range(IBS)]

    with tile.TileContext(nc) as tc:
        with (
            tc.tile_pool(name="const", bufs=1) as const,
            tc.tile_pool(name="persist", bufs=1) as persist,
        ):
            ident = const.tile([128, 128], bf16)
            eps_t = const.tile([128, 1], f32)
            ebias_t = const.tile([128, 1], f32)
            g_t = const.tile([128, 64], bf16)

            qkT = persist.tile([64, 5, T], bf16)
            # staged qkv projection (f32), persistent so v-side copies can
            # batch over 4 t-tiles at a time
            qkv_sp = persist.tile([128, TT, NQKV], f32, name="qkv_sp")
            # v variants (a: even head, 68 cols, den col 64;
            #             b: odd head, 128 cols, den col 32, v at 64:128)
            v8a = persist.tile([128, TT, 128], f8, name="v8a")
            v8b = persist.tile([128, TT, 128], f8, name="v8b")
            vr8a = persist.tile([128, TT, 128], f8, name="vr8a")
            vr8b = persist.tile([128, TT, 128], f8, name="vr8b")
            vba = persist.tile([128, TT, 68], bf16, name="vba")
            vbb = persist.tile([128, TT, 128], bf16, name="vbb")
            # ctx in fp8 + residual: [128, pair, slot(ctx8,cr8), T]
            ctxc8 = persist.tile([128, 2, 2, T], f8, name="ctxc8")
            dbc = [persist.tile([128, T], bf16, name=f"dbc{p}")
                   for p in range(2)]
            woc_sb = persist.tile([128, 2, 2, D], f8, name="woc_sb")
            mpat_sb = persist.tile([128, n_pat, 512], bf16, name="mpat_sb")

            # ---------- phase 1: project qkv (fp8 3-term DR), norm+rope ----
            with (
                tc.tile_pool(name="p1w", bufs=1) as p1w,
                tc.tile_pool(name="p1x", bufs=5) as p1x,
                tc.tile_pool(name="p1t", bufs=5) as p1t,
                tc.tile_pool(name="ps1b", bufs=3, space="PSUM") as ps1b,
                tc.tile_pool(name="ps1c", bufs=2, space="PSUM") as ps1c,
            ):
                wc_sb = p1w.tile([128, CC, 2, NQKV], f8)
                ctab = p1w.tile([128, TT, 2, HD], bf16, name="ctab")
                ctab_r = cosa_d.rearrange("(tt p) (u d) -> p tt u d",
                                          p=128, u=2)
                stab = p1w.tile([128, TT, 2, HD], bf16, name="stab")
                stab_r = sina_d.rearrange("(tt p) (u d) -> p tt u d",
                                          p=128, u=2)
                # x/qkv-weight/table loads first; phase-2-only tensors after
                for wq in range(4):
                    sl = slice(wq * 4, (wq + 1) * 4)
                    nc.scalar.dma_start(out=wc_sb[:, sl], in_=wc_d[:, sl])
                nc.scalar.dma_start(out=ctab, in_=ctab_r)
                nc.scalar.dma_start(out=stab, in_=stab_r)
                # constants / aux columns after the DMA triggers
                make_identity(nc, ident)
                nc.vector.memset(eps_t, EPS * G * G)
                nc.vector.memset(ebias_t, -ESH * float(np.log(2.0)))
                nc.vector.memset(g_t, 1.0 / G)
                nc.gpsimd.memset(v8a[:, :, 64:128], 0.0)
                nc.gpsimd.memset(v8a[:, :, 64:65], 1.0)
                nc.vector.memset(v8b[:, :, 0:64], 0.0)
                nc.vector.memset(v8b[:, :, 32:33], 1.0)
                nc.gpsimd.memset(vr8a[:, :, 64:128], 0.0)
                nc.vector.memset(vr8b[:, :, 0:64], 0.0)
                nc.gpsimd.memset(vba[:, :, 64:68], 0.0)
                nc.gpsimd.memset(vba[:, :, 64:65], 1.0)
                nc.vector.memset(vbb[:, :, 0:64], 0.0)
                nc.vector.memset(vbb[:, :, 32:33], 1.0)

                pending_qb = []

                def flush_qb():
                    for qb_p, tt_p in pending_qb:
                        psqt = ps1c.tile([64, 5, 128], bf16, tag="psqt")
                        for u in range(5):
                            nc.tensor.transpose(psqt[:, u, :], qb_p[:, u, :],
                                                ident)
                        nc.scalar.copy(
                            qkT[:, :, tt_p * 128:(tt_p + 1) * 128], psqt
                        )
                    pending_qb.clear()

                def emit_v_batch(g):
                    # batched fp8/bf16 v staging for t-tiles 4g..4g+3
                    ts4 = slice(4 * g, 4 * g + 4)
                    vf = qkv_sp[:, ts4, NQK:NQKV]
                    nc.gpsimd.tensor_copy(v8a[:, ts4, 0:64], vf)
                    nc.gpsimd.tensor_copy(v8b[:, ts4, 64:128], vf)
                    nc.gpsimd.tensor_sub(vr8a[:, ts4, 0:64], vf,
                                         v8a[:, ts4, 0:64])
                    nc.gpsimd.tensor_copy(vr8b[:, ts4, 64:128],
                                          vr8a[:, ts4, 0:64])
                    nc.gpsimd.tensor_copy(vba[:, ts4, 0:64], vf)
                    nc.gpsimd.tensor_copy(vbb[:, ts4, 64:128], vf)

                for tt in range(TT):
                    xcr = p1x.tile([128, CC, 2, 128], f8, tag="xcr")
                    nc.sync.dma_start(out=xcr, in_=xc_d[:, tt])
                    psqkv = ps1b.tile([128, NQKV], f32, tag="psqkv")
                    # transposes of the PREVIOUS tt go first so the PE
                    # never waits on the rope chain
                    flush_qb()
                    # main: x8 (slot1) x W8 (slot0), cc-pairs as DR groups
                    for c2 in range(CC // 2):
                        mmr(psqkv, xcr[:, 2 * c2:2 * c2 + 2, 1, :],
                            wc_sb[:, 2 * c2:2 * c2 + 2, 0, :],
                            start=(c2 == 0), stop=False, perf_mode=DR)
                    # corr: (xr8, x8) x (W8, Wr8) = xr8.W8 + x8.Wr8
                    for cc in range(CC):
                        mmr(psqkv, xcr[:, cc, :, :], wc_sb[:, cc, :, :],
                            start=False, stop=(cc == CC - 1), perf_mode=DR)

                    # single fast staging copy frees the psum ring quickly
                    nc.vector.tensor_copy(qkv_sp[:, tt], psqkv)
                    qk5 = qkv_sp[:, tt, 0:NQK].rearrange(
                        "p (u d) -> p u d", u=5)

                    # rope FIRST on the raw (G-scaled) projections, the
                    # rms-norm scalar lands at the end: rope commutes with
                    # the per-(token,unit) rinv, so the sqrt chain computes
                    # concurrently instead of gating the whole chain
                    sq = p1t.tile([128, 5, HD], f32, tag="sq")
                    nc.scalar.activation(sq, qk5, AF.Square)
                    ssum = p1t.tile([128, 5, 1], f32, tag="ssum")
                    nc.vector.reduce_sum(ssum, sq, axis=AX.X)
                    rinv = p1t.tile([128, 5, 1], f32, tag="rinv")
                    nc.scalar.activation(rinv, ssum, AF.Sqrt,
                                         bias=eps_t[:, 0:1], scale=1.0 / HD)
                    nc.vector.reciprocal(rinv, rinv)
                    cq = ctab[:, tt, 0:1, :].to_broadcast([128, 4, HD])
                    qr = p1t.tile([128, 5, HD], bf16, tag="qr")
                    nc.vector.tensor_mul(qr[:, 0:4, :], qk5[:, 0:4, :], cq)
                    nc.vector.tensor_mul(qr[:, 4:5, :], qk5[:, 4:5, :],
                                         ctab[:, tt, 1:2, :])
                    qrot = p1t.tile([128, 5, HD], bf16, tag="qrot")
                    nc.gpsimd.tensor_mul(
                        qrot[:, 0:4, 0:32], qk5[:, 0:4, 32:64],
                        stab[:, tt, 0:1, 0:32].to_broadcast([128, 4, 32]),
                    )
                    nc.gpsimd.tensor_mul(
                        qrot[:, 0:4, 32:64], qk5[:, 0:4, 0:32],
                        stab[:, tt, 0:1, 32:64].to_broadcast([128, 4, 32]),
                    )
                    nc.gpsimd.tensor_mul(
                        qrot[:, 4:5, 0:32], qk5[:, 4:5, 32:64],
                        stab[:, tt, 1:2, 0:32],
                    )
                    nc.gpsimd.tensor_mul(
                        qrot[:, 4:5, 32:64], qk5[:, 4:5, 0:32],
                        stab[:, tt, 1:2, 32:64],
                    )
                    qa = p1t.tile([128, 5, HD], bf16, tag="qa")
                    nc.vector.tensor_add(qa, qr, qrot)
                    qb = p1t.tile([128, 5, HD], bf16, tag="qb")
                    nc.vector.tensor_mul(qb, qa,
                                         rinv.to_broadcast([128, 5, HD]))
                    pending_qb.append((qb, tt))
                    if tt % 4 == 3:
                        emit_v_batch(tt // 4)
                    if tt == 10:
                        # phase-2-only tensors ride in after the x stream
                        # has mostly landed (the 360GB/s DMA roof gates
                        # phase 1)
                        nc.scalar.dma_start(out=woc_sb, in_=woc_d[:])
                        nc.scalar.dma_start(
                            out=mpat_sb, in_=mpat_d.rearrange("n p f -> p n f")
                        )
                flush_qb()

            # ---------- phase 2: attention + den + out-proj ----------
            with (
                tc.tile_pool(name="p2e8", bufs=14) as p2e8,
                tc.tile_pool(name="p2eb", bufs=7) as p2eb,
                tc.tile_pool(name="p2e", bufs=4) as p2e,
                tc.tile_pool(name="p2o", bufs=2) as p2o,
                tc.tile_pool(name="ps2s", bufs=3, space="PSUM") as ps2s,
                tc.tile_pool(name="ps2c", bufs=2, space="PSUM") as ps2c,
            ):
                inv_sqrt_d = float(1.0 / np.sqrt(HD))
                ot_n = [0]
                act_free = [False]

                def out_proj_quarter(ib, t4, tail=False):
                    tail = tail or act_free[0]
                    # 3-term fp8 DR out-proj; psum slots come from the shared
                    # score ring, staging copies alternate DVE / ACT
                    tt = ib * 4 + t4
                    tw = slice(tt * 128, (tt + 1) * 128)
                    ot = p2o.tile([128, D], bf16, tag="ot")
                    if tail:
                        # the score ring is free once the last exps are in
                        # flight: 2 cb per 2-bank slot, wide staging copies
                        for half4 in range(2):
                            pso = ps2s.tile([128, 2, 512], f32, tag="pss")
                            for hcb in range(2):
                                cb = half4 * 2 + hcb
                                cw = slice(cb * 512, (cb + 1) * 512)
                                mmr(pso[:, hcb, :], ctxc8[:, :, 0, tw],
                                    woc_sb[:, :, 1, cw],
                                    start=True, stop=False, perf_mode=DR)
                                for p in range(2):
                                    mmr(pso[:, hcb, :], ctxc8[:, p, :, tw],
                                        woc_sb[:, p, :, cw],
                                        start=False, stop=(p == 1),
                                        perf_mode=DR)
                            eng = (nc.scalar.copy if ot_n[0] % 2 == 1
                                   else nc.vector.tensor_copy)
                            ot_n[0] += 1
                            eng(ot[:, half4 * 1024:(half4 + 1) * 1024]
                                .rearrange("p (a b) -> p a b", a=2), pso)
                    else:
                        for cb in range(4):
                            cw = slice(cb * 512, (cb + 1) * 512)
                            pso = ps2c.tile([128, 512], f32, tag="psc")
                            mmr(pso, ctxc8[:, :, 0, tw], woc_sb[:, :, 1, cw],
                                start=True, stop=False, perf_mode=DR)
                            for p in range(2):
                                mmr(pso, ctxc8[:, p, :, tw],
                                    woc_sb[:, p, :, cw],
                                    start=False, stop=(p == 1), perf_mode=DR)
                            eng = (nc.scalar.copy
                                   if (act_free[0] and ot_n[0] % 2 == 1)
                                   else nc.vector.tensor_copy)
                            ot_n[0] += 1
                            eng(ot[:, cw], pso)
                    nc.sync.dma_start(out=out_d[tt * 128:(tt + 1) * 128, :],
                                      in_=ot)

                def den_pair(ib, pair, pe, po, den_sb):
                    # reciprocal of both heads' denominators -> broadcast
                    # across partitions with K=1 matmuls (value 1/G folds
                    # away the v-side G prescale) -> normalize ctx straight
                    # from psum, emit fp8 ctx + fp8 residual for out-proj.
                    iw = slice(ib * 512, (ib + 1) * 512)
                    with nc.allow_low_precision(
                        reason="1/den in bf16: 0.4% on softmax scale is "
                               "well inside the 2e-2 tolerance"
                    ):
                        nc.vector.reciprocal(den_sb[32:33, :], po[32:33, :])
                    pdb = ps2s.tile([128, 2, 512], f32, tag="pss")
                    mmr(pdb[0:64, 0, :], g_t[64:65, :], den_sb[64:65, :],
                        start=True, stop=True)
                    mmr(pdb[64:128, 0, :], g_t[32:33, :], den_sb[32:33, :],
                        start=True, stop=True)
                    nc.vector.tensor_copy(dbc[pair][:, iw], pdb[:, 0, :])
                    ctx_n = p2e.tile([128, 512], bf16, tag="ctx_n")
                    nc.vector.tensor_mul(ctx_n[0:64, :], pe[0:64, :],
                                         dbc[pair][0:64, iw])
                    nc.vector.tensor_mul(ctx_n[64:128, :], po[64:128, :],
                                         dbc[pair][64:128, iw])
                    nc.gpsimd.tensor_copy(ctxc8[:, pair, 0, iw], ctx_n)
                    nc.gpsimd.tensor_sub(ctxc8[:, pair, 1, iw], ctx_n,
                                         ctxc8[:, pair, 0, iw])

                def emit_scores(h, ib):
                    iw = slice(ib * 512, (ib + 1) * 512)
                    pairs, diag = ib_pairs_diag[ib]
                    et8s = []
                    for (j0, j1) in pairs:
                        pss = ps2s.tile([128, 2, 512], f32, tag="pss")
                        for half, jb in enumerate((j0, j1)):
                            mmr(pss[:, half, :],
                                qkT[:, 4, jb * 128:(jb + 1) * 128],
                                qkT[:, h, iw],
                                start=True, stop=True)
                        et8 = p2e8.tile([128, 2, 512], f8, tag="et8")
                        nc.scalar.activation(et8, pss, AF.Exp,
                                             scale=inv_sqrt_d,
                                             bias=ebias_t[:, 0:1])
                        et8s.append(et8)
                    etbs = []
                    for n in range(0, len(diag), 2):
                        dgrp = diag[n:n + 2]
                        pss = ps2s.tile([128, 2, 512], f32, tag="pss")
                        pre_g = 512
                        for half, (jb, pat) in enumerate(dgrp):
                            pre = pat_prefix[pat] if pat is not None else 0
                            pre_g = min(pre_g, pre)
                            mmr(pss[:, half, pre:512],
                                qkT[:, 4, jb * 128:(jb + 1) * 128],
                                qkT[:, h, ib * 512 + pre:(ib + 1) * 512],
                                start=True, stop=True)
                        etb = p2eb.tile([128, 2, 512], bf16, tag="etb")
                        nc.scalar.activation(etb[:, :, pre_g:512],
                                             pss[:, :, pre_g:512],
                                             AF.Exp, scale=inv_sqrt_d,
                                             bias=ebias_t[:, 0:1])
                        meng = (nc.vector.tensor_mul if ib == 0
                                else nc.gpsimd.tensor_mul)
                        for half, (jb, pat) in enumerate(dgrp):
                            if pat is None:
                                continue
                            pre = pat_prefix[pat]
                            meng(
                                etb[:, half, pre:512],
                                etb[:, half, pre:512],
                                mpat_sb[:, pat, pre:512],
                            )
                        etbs.append((etb, dgrp))
                    return et8s, etbs

                psc_of = {}
                den_sb_of = {}
                wo_queue = []

                def emit_pv(h, ib, et8s, etbs):
                    pairs, diag = ib_pairs_diag[ib]
                    pair, sub = divmod(h, 2)
                    psc = ps2c.tile([128, 512], f32, tag="psc")
                    psc_of[h, ib] = psc
                    if sub == 0:
                        ctx_out = psc
                        ctx_bout = psc[0:68, :]
                        va8, vr8, vab = v8a, vr8a, vba
                    else:
                        ctx_out = psc
                        ctx_bout = psc
                        va8, vr8, vab = v8b, vr8b, vbb
                    n_mm = 2 * len(pairs) + len(diag)
                    k = 0
                    for n, (j0, j1) in enumerate(pairs):
                        b2 = j0 // 2
                        assert j1 == j0 + 1 and j0 % 2 == 0
                        for vv in (va8, vr8):
                            mmr(ctx_out, vv[:, 2 * b2:2 * b2 + 2, :],
                                et8s[n], start=(k == 0),
                                stop=(k == n_mm - 1), perf_mode=DR)
                            k += 1
                    for etb, dgrp in etbs:
                        for half, (jb, pat) in enumerate(dgrp):
                            pre = (pat_prefix[pat]
                                   if pat is not None else 0)
                            mmr(ctx_bout[:, pre:512], vab[:, jb, :],
                                etb[:, half, pre:512],
                                start=(k == 0), stop=(k == n_mm - 1))
                            k += 1
                    if sub == 0:
                        den_sb = p2e.tile([65, 512], bf16, tag="den_sb")
                        with nc.allow_low_precision(
                            reason="1/den in bf16, inside 2e-2 tol"
                        ):
                            nc.vector.reciprocal(den_sb[64:65, :],
                                                 psc[64:65, :])
                        den_sb_of[pair, ib] = den_sb
                    else:
                        den_pair(ib, pair, psc_of[h - 1, ib], psc,
                                 den_sb_of.pop((pair, ib)))
                        for _ in range(2):
                            if wo_queue:
                                out_proj_quarter(*wo_queue.pop(0))

                # software pipeline: scores/exps of unit n overlap the PV /
                # den / out-proj of unit n-1 so the in-order PE queue never
                # parks on an exp wait.  Unit order: ib0 zipped with ib3
                # (tiny + huge complement each other and cover the phase-1
                # seam), then ib2, then ib1.  Completed i-blocks enqueue
                # out-proj quarters, popped two per den event.
                # pair-granular order: even/odd heads of a pair stay
                # adjacent (the 2-slot psc ring frees at each den event)
                units = [(h, ib) for ib in (0, 2, 3, 1)
                         for h in range(H_LOC)]
                done_cnt = {}
                prev = None
                for n, (h, ib) in enumerate(units):
                    act_free[0] = n >= 12
                    ets = emit_scores(h, ib)
                    if prev is not None:
                        emit_pv(prev[0], prev[1], *prev[2])
                        pib = prev[1]
                        done_cnt[pib] = done_cnt.get(pib, 0) + 1
                        if done_cnt[pib] == H_LOC:
                            wo_queue.extend((pib, t4) for t4 in range(4))
                    prev = (h, ib, ets)
                act_free[0] = True
                emit_pv(prev[0], prev[1], *prev[2])
                wo_queue.extend((prev[1], t4) for t4 in range(4))
                while wo_queue:
                    out_proj_quarter(*wo_queue.pop(0), tail=True)

    _split_multiwaits(nc)
    return nc


_CACHE = {}


def _get_program(mask_key, status, n_pat, pat_prefix):
    if mask_key not in _CACHE:
        _CACHE[mask_key] = _build_program(status, n_pat, pat_prefix)
    return _CACHE[mask_key]


def _f8_pair(a):
    """Split a into (fp8 value, fp8 residual)."""
    hi = a.astype(F8)
    lo = (a - hi.astype(np.float32)).astype(F8)
    return hi, lo


def _prepare(x, mask, cos, sin, W_query, W_key, W_value, W_out,
             q_scale, k_scale):
    """Host-side prep: transpose+fp8-split x, fold scales into rope tables,
    shard + fp8-split weights, classify the mask."""
    cos = np.asarray(cos, dtype=np.float32)
    sin = np.asarray(sin, dtype=np.float32)
    W_query = np.asarray(W_query, dtype=np.float32)
    W_key = np.asarray(W_key, dtype=np.float32)
    W_value = np.asarray(W_value, dtype=np.float32)
    W_out = np.asarray(W_out, dtype=np.float32)
    q_scale = np.asarray(q_scale, dtype=np.float32)
    k_scale = np.asarray(k_scale, dtype=np.float32)
    mask = np.asarray(mask)

    xT = np.asarray(x, dtype=np.float32).reshape(T, D).T  # [D, T]
    x8, xr8 = _f8_pair(xT)
    # xc [128, TT, CC, 2, 128]: slot0 = xr8, slot1 = x8
    xv = np.stack([xr8, x8], axis=0).reshape(2, CC, 128, TT, 128)
    xc = np.ascontiguousarray(xv.transpose(2, 3, 1, 0, 4))

    # rope = qn*cos' + shuffle32(qn)*sin' with the rotate-half signs and the
    # post-norm q/k scales folded into the tables
    def tables(scale):
        perm = np.concatenate([scale[HD // 2:], scale[:HD // 2]])
        c = (cos * scale[None, :]).astype(np.float32)
        s = (sin * perm[None, :]).astype(np.float32)
        s[:, :HD // 2] *= -1.0
        return c, s

    cq, sq_t = tables(q_scale)
    ck, sk_t = tables(k_scale)
    cosa = np.ascontiguousarray(
        np.concatenate([cq, ck], axis=1).astype(BF16)
    )
    sina = np.ascontiguousarray(
        np.concatenate([sq_t, sk_t], axis=1).astype(BF16)
    )

    status, patterns, prefixes = _classify_mask(mask)
    nc = _get_program(mask.tobytes(), status, patterns.shape[0], prefixes)
    patterns_bf = patterns.astype(BF16)

    in_maps = []
    for c in range(N_CORES):
        qcols = slice(c * H_LOC * HD, (c + 1) * H_LOC * HD)
        kvcols = slice(c * HD, (c + 1) * HD)
        wqkv = np.concatenate(
            [W_query[:, qcols], W_key[:, kvcols], W_value[:, kvcols]], axis=1
        ) * G
        w8, wr8 = _f8_pair(wqkv)  # [D, 384]
        # wc [128, CC, 2, 384]: slot0 = W8, slot1 = Wr8
        wc = np.ascontiguousarray(
            np.stack([w8, wr8], axis=0).reshape(2, CC, 128, NQKV_HOST)
            .transpose(2, 1, 0, 3)
        )
        woG = W_out[qcols, :] * G  # [256, D]
        wo8, wor8 = _f8_pair(woG)
        # woc [128, 2(pair), 2(slot), D]: slot0 = wor8, slot1 = wo8
        woc = np.ascontiguousarray(
            np.stack([wor8, wo8], axis=0).reshape(2, 2, 128, D)
            .transpose(2, 1, 0, 3)
        )
        in_maps.append({
            "xc": xc,
            "wc": wc,
            "woc": woc,
            "cosa": cosa, "sina": sina,
            "mpat": patterns_bf,
        })
    return nc, in_maps


NQKV_HOST = H_LOC * HD + 2 * HD


def kernel(x, mask, cos, sin, W_query, W_key, W_value, W_out,
           q_scale, k_scale):
    out_dtype = np.asarray(x).dtype
    nc, in_maps = _prepare(x, mask, cos, sin, W_query, W_key, W_value,
                           W_out, q_scale, k_scale)

    from concourse.bass_utils import run_bass_kernel_spmd

    res = run_bass_kernel_spmd(nc, in_maps, list(range(N_CORES)))
    acc = res.results[0]["out"].astype(np.float32)
    for c in range(1, N_CORES):
        acc = acc + res.results[c]["out"].astype(np.float32)
    acc *= 1.0 / G  # compensate the W_out prescale
    return acc.reshape(1, T, D).astype(out_dtype)


# revision 4
# speedup vs baseline: 1.1663x; 1.0024x over previous
"""Grouped-Query Attention kernel v2 for 8 Trainium2 NeuronCores.

Reference model: x[1,2048,2048] -> Q(32 heads x 64) / K,V(8 kv heads x 64),
per-head RMS-norm(Q,K) + RoPE, causal softmax attention, out-projection.

Sharding (tensor-parallel over heads): core c owns Q heads 4c..4c+3 and KV
head c (its GQA group) and W_out rows [256c : 256c+256).  Each core computes
a full-shape partial output; the host sums the 8 partials.

v2 speedups over the 201us baseline:
  - x is transposed on the HOST (no on-chip PE transposes of x)
  - QKV projection runs in fp8 DoubleRow mode (0.5 cyc/col) with 3-term
    residual compensation:  xW ~= x8.W8 + x8.Wr8 + xr8.W8  where
    a = a8 + ar8 splits every operand into fp8 value + fp8 residual.
    The two correction terms share one DoubleRow matmul via its 2 k-groups.
  - PV runs in fp8 DoubleRow over jb-PAIRS: exp outputs e4m3 directly
    (scaled 2^-4; numerator and denominator share the quantized weights so
    softmax stays exactly normalized); v uses fp8 + fp8-residual chains.
  - out-projection is fp8 DoubleRow 3-term (ctx8/cr8 x wo8/wor8).
  - scores stay bf16 (fp8 q/k costs 1.8e-2 of the 2e-2 error budget).
  - diagonal (causally masked) tiles stay bf16 end-to-end: bf16 exp with
    prefix-trimmed windows, 2x-mode DVE pattern multiplies, bf16 PV.
  - weights/x prescaled by G=32 on host to keep fp8 in its normal range;
    compensated exactly via a 1/G ones-column in the den broadcast and a
    final 1/G on the host.
"""

import numpy as np
import ml_dtypes

BF16 = ml_dtypes.bfloat16
F8 = ml_dtypes.float8_e4m3fn

T = 2048
D = 2048
NUM_HEADS = 32
NUM_KV = 8
HD = 64
N_CORES = 8
H_LOC = NUM_HEADS // N_CORES  # 4 q heads per core
EPS = 1e-6
G = 32.0        # fp8 prescale on W_qkv and W_out
ESH = 4         # exp output scaled by 2^-ESH to fit e4m3

TT = T // 128   # 16 t-tiles of 128 rows
CC = D // 128   # 16 contraction chunks
IBS = T // 512  # 4 i-blocks of 512 query positions
JBS = T // 128  # 16 j-blocks of 128 key positions

KEEP = "keep"
SKIP = "skip"
AFFINE = "affine"  # kept for test.py compat; no longer produced


def _classify_mask(mask: np.ndarray):
    """Per (ib, jb) scoresT tile: KEEP / SKIP / ('pat', idx) with deduped
    multiplicative keep-masks in S^T (j, i) layout.  A causal mask yields
    just 4 distinct edge patterns."""
    keep = ~mask
    status = [[KEEP] * JBS for _ in range(IBS)]
    pat_index: dict[bytes, int] = {}
    pats: list[np.ndarray] = []
    for ib in range(IBS):
        for jb in range(JBS):
            sub = keep[ib * 512:(ib + 1) * 512, jb * 128:(jb + 1) * 128]
            if sub.all():
                status[ib][jb] = KEEP
            elif not sub.any():
                status[ib][jb] = SKIP
            else:
                key = sub.tobytes()
                if key not in pat_index:
                    pat_index[key] = len(pats)
                    pats.append(sub.T.astype(np.float32))  # [128 j, 512 i]
                status[ib][jb] = ("pat", pat_index[key])
    patterns = (
        np.stack(pats) if pats else np.zeros((1, 128, 512), dtype=np.float32)
    )
    # leading i-columns that are fully masked in each pattern: the score
    # matmul / exp / PV only need the live suffix
    prefixes = []
    for p in patterns:
        alive = p.any(axis=0)
        prefixes.append(int(alive.argmax()) if alive.any() else 512)
    return status, patterns, prefixes


def _split_multiwaits(nc):
    """walrus in this container accepts only ONE sync-wait per instruction;
    hoist extra waits onto preceding same-engine NoOps (program order on the
    engine queue preserves the gating)."""
    import bass_rust
    from concourse import mybir

    n_fixed = 0
    for fn in nc.m.functions:
        for bb in fn.blocks:
            out = []
            for ins in bb.instructions:
                si = ins.sync_info
                if si is not None and si.on_wait and len(si.on_wait) > 1:
                    waits = list(si.on_wait)
                    ups = list(si.on_update) if si.on_update else []
                    for k, w in enumerate(waits[:-1]):
                        nop = mybir.InstNoOp(
                            name=f"{ins.name}-wnop{k}", ins=[], outs=[]
                        )
                        nop.engine = ins.engine
                        nop.sync_info = bass_rust.SyncInfo(
                            on_wait=[w], on_update=[]
                        )
                        out.append(nop)
                    ins.sync_info = bass_rust.SyncInfo(
                        on_wait=[waits[-1]], on_update=ups
                    )
                    n_fixed += 1
                out.append(ins)
            bb.instructions = out
    return n_fixed


def _plan_jbs(status, ib):
    """Split live jbs of an i-block into DR pairs (full tiles) and a bf16
    diag list [(jb, prefix)]."""
    full = [jb for jb in range(JBS) if status[ib][jb] == KEEP]
    diag = [(jb, st[1]) for jb in range(JBS)
            if isinstance(st := status[ib][jb], tuple)]
    if len(full) % 2:  # defensive: odd full count -> route one via diag path
        diag.append((full.pop(), None))
    pairs = [(full[2 * p], full[2 * p + 1]) for p in range(len(full) // 2)]
    return pairs, diag


def _build_program(status, n_pat, pat_prefix):
    import concourse.bass as bass
    import concourse.mybir as mybir
    import concourse.tile as tile
    from concourse.masks import make_identity

    f32 = mybir.dt.float32
    bf16 = mybir.dt.bfloat16
    f8 = mybir.dt.float8e4
    AX = mybir.AxisListType
    AF = mybir.ActivationFunctionType
    DR = mybir.MatmulPerfMode.DoubleRow

    nc = bass.Bass("TRN2", num_devices=N_CORES)
    # x: [128, TT, CC, 2, 128]  slot0 = xr8, slot1 = x8 (fp8, G-free)
    xc_d = nc.declare_dram_parameter("xc", [128, TT, CC, 2, 128], f8,
                                     isOutput=False)
    # W_qkv: [128, CC, 2, 384]  slot0 = W8, slot1 = Wr8 (fp8, xG)
    wc_d = nc.declare_dram_parameter("wc", [128, CC, 2, 384], f8,
                                     isOutput=False)
    # W_out: [128, 2(pair), 2(slot), D]  slot0 = wor8, slot1 = wo8 (fp8, xG)
    woc_d = nc.declare_dram_parameter("woc", [128, 2, 2, D], f8,
                                      isOutput=False)
    # rope tables, 2 units (q, k), scales folded in
    cosa_d = nc.declare_dram_parameter("cosa", [T, 2 * HD], bf16,
                                       isOutput=False)
    sina_d = nc.declare_dram_parameter("sina", [T, 2 * HD], bf16,
                                       isOutput=False)
    mpat_d = nc.declare_dram_parameter(
        "mpat", [n_pat, 128, 512], bf16, isOutput=False
    )
    out_d = nc.declare_dram_parameter("out", [T, D], bf16, isOutput=True)

    NQKV = H_LOC * HD + 2 * HD  # 384: q heads, then k, then v
    NQK = (H_LOC + 1) * HD      # 320: q heads + k (norm/rope batch)

    mmr = nc.tensor.matmul
    ib_pairs_diag = [_plan_jbs(status, ib) for ib in range(IBS)]

    with tile.TileContext(nc) as tc:
        with (
            tc.tile_pool(name="const", bufs=1) as const,
            tc.tile_pool(name="persist", bufs=1) as persist,
        ):
            ident = const.tile([128, 128], bf16)
            eps_t = const.tile([128, 1], f32)
            ebias_t = const.tile([128, 1], f32)
            g_t = const.tile([128, 64], bf16)

            qkT = persist.tile([64, 5, T], bf16)
            # staged qkv projection (f32), persistent so v-side copies can
            # batch over 4 t-tiles at a time
            qkv_sp = persist.tile([128, TT, NQKV], f32, name="qkv_sp")
            # v variants (a: even head, 68 cols, den col 64;
            #             b: odd head, 128 cols, den col 32, v at 64:128)
            v8a = persist.tile([128, TT, 128], f8, name="v8a")
            v8b = persist.tile([128, TT, 128], f8, name="v8b")
            vr8a = persist.tile([128, TT, 128], f8, name="vr8a")
            vr8b = persist.tile([128, TT, 128], f8, name="vr8b")
            vba = persist.tile([128, TT, 68], bf16, name="vba")
            vbb = persist.tile([128, TT, 128], bf16, name="vbb")
            # ctx in fp8 + residual: [128, pair, slot(ctx8,cr8), T]
            ctxc8 = persist.tile([128, 2, 2, T], f8, name="ctxc8")
            dbc = [persist.tile([128, T], bf16, name=f"dbc{p}")
                   for p in range(2)]
            woc_sb = persist.tile([128, 2, 2, D], f8, name="woc_sb")
            mpat_sb = persist.tile([128, n_pat, 512], bf16, name="mpat_sb")

            # ---------- phase 1: project qkv (fp8 3-term DR), norm+rope ----
            with (
                tc.tile_pool(name="p1w", bufs=1) as p1w,
                tc.tile_pool(name="p1x", bufs=5) as p1x,
                tc.tile_pool(name="p1t", bufs=5) as p1t,
                tc.tile_pool(name="ps1b", bufs=3, space="PSUM") as ps1b,
                tc.tile_pool(name="ps1c", bufs=2, space="PSUM") as ps1c,
            ):
                wc_sb = p1w.tile([128, CC, 2, NQKV], f8)
                ctab = p1w.tile([128, TT, 2, HD], bf16, name="ctab")
                ctab_r = cosa_d.rearrange("(tt p) (u d) -> p tt u d",
                                          p=128, u=2)
                stab = p1w.tile([128, TT, 2, HD], bf16, name="stab")
                stab_r = sina_d.rearrange("(tt p) (u d) -> p tt u d",
                                          p=128, u=2)
                # x/qkv-weight/table loads first; phase-2-only tensors after
                for wq in range(4):
                    sl = slice(wq * 4, (wq + 1) * 4)
                    nc.scalar.dma_start(out=wc_sb[:, sl], in_=wc_d[:, sl])
                nc.scalar.dma_start(out=ctab, in_=ctab_r)
                nc.scalar.dma_start(out=stab, in_=stab_r)
                # constants / aux columns after the DMA triggers
                make_identity(nc, ident)
                nc.vector.memset(eps_t, EPS * G * G)
                nc.vector.memset(ebias_t, -ESH * float(np.log(2.0)))
                nc.vector.memset(g_t, 1.0 / G)
                nc.gpsimd.memset(v8a[:, :, 64:128], 0.0)
                nc.gpsimd.memset(v8a[:, :, 64:65], 1.0)
                nc.vector.memset(v8b[:, :, 0:64], 0.0)
                nc.vector.memset(v8b[:, :, 32:33], 1.0)
                nc.gpsimd.memset(vr8a[:, :, 64:128], 0.0)
                nc.vector.memset(vr8b[:, :, 0:64], 0.0)
                nc.gpsimd.memset(vba[:, :, 64:68], 0.0)
                nc.gpsimd.memset(vba[:, :, 64:65], 1.0)
                nc.vector.memset(vbb[:, :, 0:64], 0.0)
                nc.vector.memset(vbb[:, :, 32:33], 1.0)

                pending_qb = []

                def flush_qb():
                    for qb_p, tt_p in pending_qb:
                        psqt = ps1c.tile([64, 5, 128], bf16, tag="psqt")
                        for u in range(5):
                            nc.tensor.transpose(psqt[:, u, :], qb_p[:, u, :],
                                                ident)
                        nc.scalar.copy(
                            qkT[:, :, tt_p * 128:(tt_p + 1) * 128], psqt
                        )
                    pending_qb.clear()

                def emit_v_batch(g):
                    # batched fp8/bf16 v staging for t-tiles 4g..4g+3
                    ts4 = slice(4 * g, 4 * g + 4)
                    vf = qkv_sp[:, ts4, NQK:NQKV]
                    nc.gpsimd.tensor_copy(v8a[:, ts4, 0:64], vf)
                    nc.gpsimd.tensor_copy(v8b[:, ts4, 64:128], vf)
                    nc.gpsimd.tensor_sub(vr8a[:, ts4, 0:64], vf,
                                         v8a[:, ts4, 0:64])
                    nc.gpsimd.tensor_copy(vr8b[:, ts4, 64:128],
                                          vr8a[:, ts4, 0:64])
                    nc.gpsimd.tensor_copy(vba[:, ts4, 0:64], vf)
                    nc.gpsimd.tensor_copy(vbb[:, ts4, 64:128], vf)

                for tt in range(TT):
                    xcr = p1x.tile([128, CC, 2, 128], f8, tag="xcr")
                    nc.sync.dma_start(out=xcr, in_=xc_d[:, tt])
                    psqkv = ps1b.tile([128, NQKV], f32, tag="psqkv")
                    # transposes of the PREVIOUS tt go first so the PE
                    # never waits on the rope chain
                    flush_qb()
                    # main: x8 (slot1) x W8 (slot0), cc-pairs as DR groups
                    for c2 in range(CC // 2):
                        mmr(psqkv, xcr[:, 2 * c2:2 * c2 + 2, 1, :],
                            wc_sb[:, 2 * c2:2 * c2 + 2, 0, :],
                            start=(c2 == 0), stop=False, perf_mode=DR)
                    # corr: (xr8, x8) x (W8, Wr8) = xr8.W8 + x8.Wr8
                    for cc in range(CC):
                        mmr(psqkv, xcr[:, cc, :, :], wc_sb[:, cc, :, :],
                            start=False, stop=(cc == CC - 1), perf_mode=DR)

                    # single fast staging copy frees the psum ring quickly
                    nc.vector.tensor_copy(qkv_sp[:, tt], psqkv)
                    qk5 = qkv_sp[:, tt, 0:NQK].rearrange(
                        "p (u d) -> p u d", u=5)

                    # rope FIRST on the raw (G-scaled) projections, the
                    # rms-norm scalar lands at the end: rope commutes with
                    # the per-(token,unit) rinv, so the sqrt chain computes
                    # concurrently instead of gating the whole chain
                    sq = p1t.tile([128, 5, HD], f32, tag="sq")
                    nc.scalar.activation(sq, qk5, AF.Square)
                    ssum = p1t.tile([128, 5, 1], f32, tag="ssum")
                    nc.vector.reduce_sum(ssum, sq, axis=AX.X)
                    rinv = p1t.tile([128, 5, 1], f32, tag="rinv")
                    nc.scalar.activation(rinv, ssum, AF.Sqrt,
                                         bias=eps_t[:, 0:1], scale=1.0 / HD)
                    nc.vector.reciprocal(rinv, rinv)
                    cq = ctab[:, tt, 0:1, :].to_broadcast([128, 4, HD])
                    qr = p1t.tile([128, 5, HD], bf16, tag="qr")
                    nc.vector.tensor_mul(qr[:, 0:4, :], qk5[:, 0:4, :], cq)
                    nc.vector.tensor_mul(qr[:, 4:5, :], qk5[:, 4:5, :],
                                         ctab[:, tt, 1:2, :])
                    qrot = p1t.tile([128, 5, HD], bf16, tag="qrot")
                    nc.gpsimd.tensor_mul(
                        qrot[:, 0:4, 0:32], qk5[:, 0:4, 32:64],
                        stab[:, tt, 0:1, 0:32].to_broadcast([128, 4, 32]),
                    )
                    nc.gpsimd.tensor_mul(
                        qrot[:, 0:4, 32:64], qk5[:, 0:4, 0:32],
                        stab[:, tt, 0:1, 32:64].to_broadcast([128, 4, 32]),
                    )
                    nc.gpsimd.tensor_mul(
                        qrot[:, 4:5, 0:32], qk5[:, 4:5, 32:64],
                        stab[:, tt, 1:2, 0:32],
                    )
                    nc.gpsimd.tensor_mul(
                        qrot[:, 4:5, 32:64], qk5[:, 4:5, 0:32],
                        stab[:, tt, 1:2, 32:64],
                    )
                    qa = p1t.tile([128, 5, HD], bf16, tag="qa")
                    nc.vector.tensor_add(qa, qr, qrot)
                    qb = p1t.tile([128, 5, HD], bf16, tag="qb")
                    nc.vector.tensor_mul(qb, qa,
                                         rinv.to_broadcast([128, 5, HD]))
                    pending_qb.append((qb, tt))
                    if tt % 4 == 3:
                        emit_v_batch(tt // 4)
                    if tt == 10:
                        # phase-2-only tensors ride in after the x stream
                        # has mostly landed (the 360GB/s DMA roof gates
                        # phase 1)
                        nc.scalar.dma_start(out=woc_sb, in_=woc_d[:])
                        nc.scalar.dma_start(
                            out=mpat_sb, in_=mpat_d.rearrange("n p f -> p n f")
                        )
                flush_qb()

            # ---------- phase 2: attention + den + out-proj ----------
            with (
                tc.tile_pool(name="p2e8", bufs=14) as p2e8,
                tc.tile_pool(name="p2eb", bufs=7) as p2eb,
                tc.tile_pool(name="p2e", bufs=4) as p2e,
                tc.tile_pool(name="p2o", bufs=2) as p2o,
                tc.tile_pool(name="ps2s", bufs=3, space="PSUM") as ps2s,
                tc.tile_pool(name="ps2c", bufs=2, space="PSUM") as ps2c,
            ):
                inv_sqrt_d = float(1.0 / np.sqrt(HD))
                ot_n = [0]
                act_free = [False]

                def out_proj_quarter(ib, t4, tail=False):
                    # 3-term fp8 DR out-proj; psum slots come from the shared
                    # score ring, staging copies alternate DVE / ACT
                    tt = ib * 4 + t4
                    tw = slice(tt * 128, (tt + 1) * 128)
                    ot = p2o.tile([128, D], bf16, tag="ot")
                    if tail:
                        # the score ring is free once the last exps are in
                        # flight: 2 cb per 2-bank slot, wide staging copies
                        for half4 in range(2):
                            pso = ps2s.tile([128, 2, 512], f32, tag="pss")
                            for hcb in range(2):
                                cb = half4 * 2 + hcb
                                cw = slice(cb * 512, (cb + 1) * 512)
                                mmr(pso[:, hcb, :], ctxc8[:, :, 0, tw],
                                    woc_sb[:, :, 1, cw],
                                    start=True, stop=False, perf_mode=DR)
                                for p in range(2):
                                    mmr(pso[:, hcb, :], ctxc8[:, p, :, tw],
                                        woc_sb[:, p, :, cw],
                                        start=False, stop=(p == 1),
                                        perf_mode=DR)
                            eng = (nc.scalar.copy if ot_n[0] % 2 == 1
                                   else nc.vector.tensor_copy)
                            ot_n[0] += 1
                            eng(ot[:, half4 * 1024:(half4 + 1) * 1024]
                                .rearrange("p (a b) -> p a b", a=2), pso)
                    else:
                        for cb in range(4):
                            cw = slice(cb * 512, (cb + 1) * 512)
                            pso = ps2c.tile([128, 512], f32, tag="psc")
                            mmr(pso, ctxc8[:, :, 0, tw], woc_sb[:, :, 1, cw],
                                start=True, stop=False, perf_mode=DR)
                            for p in range(2):
                                mmr(pso, ctxc8[:, p, :, tw],
                                    woc_sb[:, p, :, cw],
                                    start=False, stop=(p == 1), perf_mode=DR)
                            eng = (nc.scalar.copy
                                   if (act_free[0] and ot_n[0] % 2 == 1)
                                   else nc.vector.tensor_copy)
                            ot_n[0] += 1
                            eng(ot[:, cw], pso)
                    nc.sync.dma_start(out=out_d[tt * 128:(tt + 1) * 128, :],
                                      in_=ot)

                def den_pair(ib, pair, pe, po, den_sb):
                    # reciprocal of both heads' denominators -> broadcast
                    # across partitions with K=1 matmuls (value 1/G folds
                    # away the v-side G prescale) -> normalize ctx straight
                    # from psum, emit fp8 ctx + fp8 residual for out-proj.
                    iw = slice(ib * 512, (ib + 1) * 512)
                    with nc.allow_low_precision(
                        reason="1/den in bf16: 0.4% on softmax scale is "
                               "well inside the 2e-2 tolerance"
                    ):
                        nc.vector.reciprocal(den_sb[32:33, :], po[32:33, :])
                    pdb = ps2s.tile([128, 2, 512], f32, tag="pss")
                    mmr(pdb[0:64, 0, :], g_t[64:65, :], den_sb[64:65, :],
                        start=True, stop=True)
                    mmr(pdb[64:128, 0, :], g_t[32:33, :], den_sb[32:33, :],
                        start=True, stop=True)
                    nc.vector.tensor_copy(dbc[pair][:, iw], pdb[:, 0, :])
                    ctx_n = p2e.tile([128, 512], bf16, tag="ctx_n")
                    nc.vector.tensor_mul(ctx_n[0:64, :], pe[0:64, :],
                                         dbc[pair][0:64, iw])
                    nc.vector.tensor_mul(ctx_n[64:128, :], po[64:128, :],
                                         dbc[pair][64:128, iw])
                    nc.gpsimd.tensor_copy(ctxc8[:, pair, 0, iw], ctx_n)
                    nc.gpsimd.tensor_sub(ctxc8[:, pair, 1, iw], ctx_n,
                                         ctxc8[:, pair, 0, iw])

                def emit_scores(h, ib):
                    iw = slice(ib * 512, (ib + 1) * 512)
                    pairs, diag = ib_pairs_diag[ib]
                    et8s = []
                    for (j0, j1) in pairs:
                        pss = ps2s.tile([128, 2, 512], f32, tag="pss")
                        for half, jb in enumerate((j0, j1)):
                            mmr(pss[:, half, :],
                                qkT[:, 4, jb * 128:(jb + 1) * 128],
                                qkT[:, h, iw],
                                start=True, stop=True)
                        et8 = p2e8.tile([128, 2, 512], f8, tag="et8")
                        nc.scalar.activation(et8, pss, AF.Exp,
                                             scale=inv_sqrt_d,
                                             bias=ebias_t[:, 0:1])
                        et8s.append(et8)
                    etbs = []
                    for n in range(0, len(diag), 2):
                        dgrp = diag[n:n + 2]
                        pss = ps2s.tile([128, 2, 512], f32, tag="pss")
                        pre_g = 512
                        for half, (jb, pat) in enumerate(dgrp):
                            pre = pat_prefix[pat] if pat is not None else 0
                            pre_g = min(pre_g, pre)
                            mmr(pss[:, half, pre:512],
                                qkT[:, 4, jb * 128:(jb + 1) * 128],
                                qkT[:, h, ib * 512 + pre:(ib + 1) * 512],
                                start=True, stop=True)
                        etb = p2eb.tile([128, 2, 512], bf16, tag="etb")
                        nc.scalar.activation(etb[:, :, pre_g:512],
                                             pss[:, :, pre_g:512],
                                             AF.Exp, scale=inv_sqrt_d,
                                             bias=ebias_t[:, 0:1])
                        meng = (nc.vector.tensor_mul if ib == 0
                                else nc.gpsimd.tensor_mul)
                        for half, (jb, pat) in enumerate(dgrp):
                            if pat is None:
                                continue
                            pre = pat_prefix[pat]
                            meng(
                                etb[:, half, pre:512],
                                etb[:, half, pre:512],
                                mpat_sb[:, pat, pre:512],
                            )
                        etbs.append((etb, dgrp))
                    return et8s, etbs

                psc_of = {}
                den_sb_of = {}
                wo_queue = []

                def emit_pv(h, ib, et8s, etbs):
                    pairs, diag = ib_pairs_diag[ib]
                    pair, sub = divmod(h, 2)
                    psc = ps2c.tile([128, 512], f32, tag="psc")
                    psc_of[h, ib] = psc
                    if sub == 0:
                        ctx_out = psc
                        ctx_bout = psc[0:68, :]
                        va8, vr8, vab = v8a, vr8a, vba
                    else:
                        ctx_out = psc
                        ctx_bout = psc
                        va8, vr8, vab = v8b, vr8b, vbb
                    n_mm = 2 * len(pairs) + len(diag)
                    k = 0
                    for n, (j0, j1) in enumerate(pairs):
                        b2 = j0 // 2
                        assert j1 == j0 + 1 and j0 % 2 == 0
                        for vv in (va8, vr8):
                            mmr(ctx_out, vv[:, 2 * b2:2 * b2 + 2, :],
                                et8s[n], start=(k == 0),
                                stop=(k == n_mm - 1), perf_mode=DR)
                            k += 1
                    for etb, dgrp in etbs:
                        for half, (jb, pat) in enumerate(dgrp):
                            pre = (pat_prefix[pat]
                                   if pat is not None else 0)
                            mmr(ctx_bout[:, pre:512], vab[:, jb, :],
                                etb[:, half, pre:512],
                                start=(k == 0), stop=(k == n_mm - 1))
                            k += 1
                    if sub == 0:
                        den_sb = p2e.tile([65, 512], bf16, tag="den_sb")
                        with nc.allow_low_precision(
                            reason="1/den in bf16, inside 2e-2 tol"
                        ):
                            nc.vector.reciprocal(den_sb[64:65, :],
                                                 psc[64:65, :])
                        den_sb_of[pair, ib] = den_sb
                    else:
                        den_pair(ib, pair, psc_of[h - 1, ib], psc,
                                 den_sb_of.pop((pair, ib)))
                        for _ in range(2):
                            if wo_queue:
                                out_proj_quarter(*wo_queue.pop(0))

                # software pipeline: scores/exps of unit n overlap the PV /
                # den / out-proj of unit n-1 so the in-order PE queue never
                # parks on an exp wait.  Unit order: ib0 zipped with ib3
                # (tiny + huge complement each other and cover the phase-1
                # seam), then ib2, then ib1.  Completed i-blocks enqueue
                # out-proj quarters, popped two per den event.
                # pair-granular order: even/odd heads of a pair stay
                # adjacent (the 2-slot psc ring frees at each den event)
                units = [(h, ib) for ib in (0, 2, 3, 1)
                         for h in range(H_LOC)]
                done_cnt = {}
                prev = None
                for n, (h, ib) in enumerate(units):
                    act_free[0] = n >= 12
                    ets = emit_scores(h, ib)
                    if prev is not None:
                        emit_pv(prev[0], prev[1], *prev[2])
                        pib = prev[1]
                        done_cnt[pib] = done_cnt.get(pib, 0) + 1
                        if done_cnt[pib] == H_LOC:
                            wo_queue.extend((pib, t4) for t4 in range(4))
                    prev = (h, ib, ets)
                act_free[0] = True
                emit_pv(prev[0], prev[1], *prev[2])
                wo_queue.extend((prev[1], t4) for t4 in range(4))
                while wo_queue:
                    out_proj_quarter(*wo_queue.pop(0), tail=True)

    _split_multiwaits(nc)
    return nc


_CACHE = {}


def _get_program(mask_key, status, n_pat, pat_prefix):
    if mask_key not in _CACHE:
        _CACHE[mask_key] = _build_program(status, n_pat, pat_prefix)
    return _CACHE[mask_key]


def _f8_pair(a):
    """Split a into (fp8 value, fp8 residual)."""
    hi = a.astype(F8)
    lo = (a - hi.astype(np.float32)).astype(F8)
    return hi, lo


def _prepare(x, mask, cos, sin, W_query, W_key, W_value, W_out,
             q_scale, k_scale):
    """Host-side prep: transpose+fp8-split x, fold scales into rope tables,
    shard + fp8-split weights, classify the mask."""
    cos = np.asarray(cos, dtype=np.float32)
    sin = np.asarray(sin, dtype=np.float32)
    W_query = np.asarray(W_query, dtype=np.float32)
    W_key = np.asarray(W_key, dtype=np.float32)
    W_value = np.asarray(W_value, dtype=np.float32)
    W_out = np.asarray(W_out, dtype=np.float32)
    q_scale = np.asarray(q_scale, dtype=np.float32)
    k_scale = np.asarray(k_scale, dtype=np.float32)
    mask = np.asarray(mask)

    xT = np.asarray(x, dtype=np.float32).reshape(T, D).T  # [D, T]
    x8, xr8 = _f8_pair(xT)
    # xc [128, TT, CC, 2, 128]: slot0 = xr8, slot1 = x8
    xv = np.stack([xr8, x8], axis=0).reshape(2, CC, 128, TT, 128)
    xc = np.ascontiguousarray(xv.transpose(2, 3, 1, 0, 4))

    # rope = qn*cos' + shuffle32(qn)*sin' with the rotate-half signs and the
    # post-norm q/k scales folded into the tables
    def tables(scale):
        perm = np.concatenate([scale[HD // 2:], scale[:HD // 2]])
        c = (cos * scale[None, :]).astype(np.float32)
        s = (sin * perm[None, :]).astype(np.float32)
        s[:, :HD // 2] *= -1.0
        return c, s

    cq, sq_t = tables(q_scale)
    ck, sk_t = tables(k_scale)
    cosa = np.ascontiguousarray(
        np.concatenate([cq, ck], axis=1).astype(BF16)
    )
    sina = np.ascontiguousarray(
        np.concatenate([sq_t, sk_t], axis=1).astype(BF16)
    )

    status, patterns, prefixes = _classify_mask(mask)
    nc = _get_program(mask.tobytes(), status, patterns.shape[0], prefixes)
    patterns_bf = patterns.astype(BF16)

    in_maps = []
    for c in range(N_CORES):
        qcols = slice(c * H_LOC * HD, (c + 1) * H_LOC * HD)
        kvcols = slice(c * HD, (c + 1) * HD)
        wqkv = np.concatenate(
            [W_query[:, qcols], W_key[:, kvcols], W_value[:, kvcols]], axis=1
        ) * G
        w8, wr8 = _f8_pair(wqkv)  # [D, 384]
        # wc [128, CC, 2, 384]: slot0 = W8, slot1 = Wr8
        wc = np.ascontiguousarray(
            np.stack([w8, wr8], axis=0).reshape(2, CC, 128, NQKV_HOST)
            .transpose(2, 1, 0, 3)
        )
        woG = W_out[qcols, :] * G  # [256, D]
        wo8, wor8 = _f8_pair(woG)
        # woc [128, 2(pair), 2(slot), D]: slot0 = wor8, slot1 = wo8
        woc = np.ascontiguousarray(
            np.stack([wor8, wo8], axis=0).reshape(2, 2, 128, D)
            .transpose(2, 1, 0, 3)
        )
        in_maps.append({
            "xc": xc,
            "wc": wc,
            "woc": woc,
            "cosa": cosa, "sina": sina,
            "mpat": patterns_bf,
        })
    return nc, in_maps


NQKV_HOST = H_LOC * HD + 2 * HD


def kernel(x, mask, cos, sin, W_query, W_key, W_value, W_out,
           q_scale, k_scale):
    out_dtype = np.asarray(x).dtype
    nc, in_maps = _prepare(x, mask, cos, sin, W_query, W_key, W_value,
                           W_out, q_scale, k_scale)

    from concourse.bass_utils import run_bass_kernel_spmd

    res = run_bass_kernel_spmd(nc, in_maps, list(range(N_CORES)))
    acc = res.results[0]["out"].astype(np.float32)
    for c in range(1, N_CORES):
        acc = acc + res.results[c]["out"].astype(np.float32)
    acc *= 1.0 / G  # compensate the W_out prescale
    return acc.reshape(1, T, D).astype(out_dtype)


# revision 5
# speedup vs baseline: 1.1664x; 1.0001x over previous
"""Grouped-Query Attention kernel v2 for 8 Trainium2 NeuronCores.

Reference model: x[1,2048,2048] -> Q(32 heads x 64) / K,V(8 kv heads x 64),
per-head RMS-norm(Q,K) + RoPE, causal softmax attention, out-projection.

Sharding (tensor-parallel over heads): core c owns Q heads 4c..4c+3 and KV
head c (its GQA group) and W_out rows [256c : 256c+256).  Each core computes
a full-shape partial output; the host sums the 8 partials.

v2 speedups over the 201us baseline:
  - x is transposed on the HOST (no on-chip PE transposes of x)
  - QKV projection runs in fp8 DoubleRow mode (0.5 cyc/col) with 3-term
    residual compensation:  xW ~= x8.W8 + x8.Wr8 + xr8.W8  where
    a = a8 + ar8 splits every operand into fp8 value + fp8 residual.
    The two correction terms share one DoubleRow matmul via its 2 k-groups.
  - PV runs in fp8 DoubleRow over jb-PAIRS: exp outputs e4m3 directly
    (scaled 2^-4; numerator and denominator share the quantized weights so
    softmax stays exactly normalized); v uses fp8 + fp8-residual chains.
  - out-projection is fp8 DoubleRow 3-term (ctx8/cr8 x wo8/wor8).
  - scores stay bf16 (fp8 q/k costs 1.8e-2 of the 2e-2 error budget).
  - diagonal (causally masked) tiles stay bf16 end-to-end: bf16 exp with
    prefix-trimmed windows, 2x-mode DVE pattern multiplies, bf16 PV.
  - weights/x prescaled by G=32 on host to keep fp8 in its normal range;
    compensated exactly via a 1/G ones-column in the den broadcast and a
    final 1/G on the host.
"""

import numpy as np
import ml_dtypes

BF16 = ml_dtypes.bfloat16
F8 = ml_dtypes.float8_e4m3fn

T = 2048
D = 2048
NUM_HEADS = 32
NUM_KV = 8
HD = 64
N_CORES = 8
H_LOC = NUM_HEADS // N_CORES  # 4 q heads per core
EPS = 1e-6
G = 32.0        # fp8 prescale on W_qkv and W_out
ESH = 4         # exp output scaled by 2^-ESH to fit e4m3

TT = T // 128   # 16 t-tiles of 128 rows
CC = D // 128   # 16 contraction chunks
IBS = T // 512  # 4 i-blocks of 512 query positions
JBS = T // 128  # 16 j-blocks of 128 key positions

KEEP = "keep"
SKIP = "skip"
AFFINE = "affine"  # kept for test.py compat; no longer produced


def _classify_mask(mask: np.ndarray):
    """Per (ib, jb) scoresT tile: KEEP / SKIP / ('pat', idx) with deduped
    multiplicative keep-masks in S^T (j, i) layout.  A causal mask yields
    just 4 distinct edge patterns."""
    keep = ~mask
    status = [[KEEP] * JBS for _ in range(IBS)]
    pat_index: dict[bytes, int] = {}
    pats: list[np.ndarray] = []
    for ib in range(IBS):
        for jb in range(JBS):
            sub = keep[ib * 512:(ib + 1) * 512, jb * 128:(jb + 1) * 128]
            if sub.all():
                status[ib][jb] = KEEP
            elif not sub.any():
                status[ib][jb] = SKIP
            else:
                key = sub.tobytes()
                if key not in pat_index:
                    pat_index[key] = len(pats)
                    pats.append(sub.T.astype(np.float32))  # [128 j, 512 i]
                status[ib][jb] = ("pat", pat_index[key])
    patterns = (
        np.stack(pats) if pats else np.zeros((1, 128, 512), dtype=np.float32)
    )
    # leading i-columns that are fully masked in each pattern: the score
    # matmul / exp / PV only need the live suffix
    prefixes = []
    for p in patterns:
        alive = p.any(axis=0)
        prefixes.append(int(alive.argmax()) if alive.any() else 512)
    return status, patterns, prefixes


def _split_multiwaits(nc):
    """walrus in this container accepts only ONE sync-wait per instruction;
    hoist extra waits onto preceding same-engine NoOps (program order on the
    engine queue preserves the gating)."""
    import bass_rust
    from concourse import mybir

    n_fixed = 0
    for fn in nc.m.functions:
        for bb in fn.blocks:
            out = []
            for ins in bb.instructions:
                si = ins.sync_info
                if si is not None and si.on_wait and len(si.on_wait) > 1:
                    waits = list(si.on_wait)
                    ups = list(si.on_update) if si.on_update else []
                    for k, w in enumerate(waits[:-1]):
                        nop = mybir.InstNoOp(
                            name=f"{ins.name}-wnop{k}", ins=[], outs=[]
                        )
                        nop.engine = ins.engine
                        nop.sync_info = bass_rust.SyncInfo(
                            on_wait=[w], on_update=[]
                        )
                        out.append(nop)
                    ins.sync_info = bass_rust.SyncInfo(
                        on_wait=[waits[-1]], on_update=ups
                    )
                    n_fixed += 1
                out.append(ins)
            bb.instructions = out
    return n_fixed


def _plan_jbs(status, ib):
    """Split live jbs of an i-block into DR pairs (full tiles) and a bf16
    diag list [(jb, prefix)]."""
    full = [jb for jb in range(JBS) if status[ib][jb] == KEEP]
    diag = [(jb, st[1]) for jb in range(JBS)
            if isinstance(st := status[ib][jb], tuple)]
    if len(full) % 2:  # defensive: odd full count -> route one via diag path
        diag.append((full.pop(), None))
    pairs = [(full[2 * p], full[2 * p + 1]) for p in range(len(full) // 2)]
    return pairs, diag


def _build_program(status, n_pat, pat_prefix):
    import concourse.bass as bass
    import concourse.mybir as mybir
    import concourse.tile as tile
    from concourse.masks import make_identity

    f32 = mybir.dt.float32
    bf16 = mybir.dt.bfloat16
    f8 = mybir.dt.float8e4
    AX = mybir.AxisListType
    AF = mybir.ActivationFunctionType
    DR = mybir.MatmulPerfMode.DoubleRow

    nc = bass.Bass("TRN2", num_devices=N_CORES)
    # x: [128, TT, CC, 2, 128]  slot0 = xr8, slot1 = x8 (fp8, G-free)
    xc_d = nc.declare_dram_parameter("xc", [128, TT, CC, 2, 128], f8,
                                     isOutput=False)
    # W_qkv: [128, CC, 2, 384]  slot0 = W8, slot1 = Wr8 (fp8, xG)
    wc_d = nc.declare_dram_parameter("wc", [128, CC, 2, 384], f8,
                                     isOutput=False)
    # W_out: [128, 2(pair), 2(slot), D]  slot0 = wor8, slot1 = wo8 (fp8, xG)
    woc_d = nc.declare_dram_parameter("woc", [128, 2, 2, D], f8,
                                      isOutput=False)
    # rope tables, 2 units (q, k), scales folded in
    cosa_d = nc.declare_dram_parameter("cosa", [T, 2 * HD], bf16,
                                       isOutput=False)
    sina_d = nc.declare_dram_parameter("sina", [T, 2 * HD], bf16,
                                       isOutput=False)
    mpat_d = nc.declare_dram_parameter(
        "mpat", [n_pat, 128, 512], bf16, isOutput=False
    )
    out_d = nc.declare_dram_parameter("out", [T, D], bf16, isOutput=True)

    NQKV = H_LOC * HD + 2 * HD  # 384: q heads, then k, then v
    NQK = (H_LOC + 1) * HD      # 320: q heads + k (norm/rope batch)

    mmr = nc.tensor.matmul
    ib_pairs_diag = [_plan_jbs(status, ib) for ib in range(IBS)]

    with tile.TileContext(nc) as tc:
        with (
            tc.tile_pool(name="const", bufs=1) as const,
            tc.tile_pool(name="persist", bufs=1) as persist,
        ):
            ident = const.tile([128, 128], bf16)
            eps_t = const.tile([128, 1], f32)
            ebias_t = const.tile([128, 1], f32)
            g_t = const.tile([128, 64], bf16)

            qkT = persist.tile([64, 5, T], bf16)
            # staged qkv projection (f32), persistent so v-side copies can
            # batch over 4 t-tiles at a time
            qkv_sp = persist.tile([128, TT, NQKV], f32, name="qkv_sp")
            # v variants (a: even head, 68 cols, den col 64;
            #             b: odd head, 128 cols, den col 32, v at 64:128)
            v8a = persist.tile([128, TT, 128], f8, name="v8a")
            v8b = persist.tile([128, TT, 128], f8, name="v8b")
            vr8a = persist.tile([128, TT, 128], f8, name="vr8a")
            vr8b = persist.tile([128, TT, 128], f8, name="vr8b")
            vba = persist.tile([128, TT, 68], bf16, name="vba")
            vbb = persist.tile([128, TT, 128], bf16, name="vbb")
            # ctx in fp8 + residual: [128, pair, slot(ctx8,cr8), T]
            ctxc8 = persist.tile([128, 2, 2, T], f8, name="ctxc8")
            dbc = [persist.tile([128, T], bf16, name=f"dbc{p}")
                   for p in range(2)]
            woc_sb = persist.tile([128, 2, 2, D], f8, name="woc_sb")
            mpat_sb = persist.tile([128, n_pat, 512], bf16, name="mpat_sb")

            # ---------- phase 1: project qkv (fp8 3-term DR), norm+rope ----
            with (
                tc.tile_pool(name="p1w", bufs=1) as p1w,
                tc.tile_pool(name="p1x", bufs=5) as p1x,
                tc.tile_pool(name="p1t", bufs=5) as p1t,
                tc.tile_pool(name="ps1b", bufs=3, space="PSUM") as ps1b,
                tc.tile_pool(name="ps1c", bufs=2, space="PSUM") as ps1c,
            ):
                wc_sb = p1w.tile([128, CC, 2, NQKV], f8)
                ctab = p1w.tile([128, TT, 2, HD], bf16, name="ctab")
                ctab_r = cosa_d.rearrange("(tt p) (u d) -> p tt u d",
                                          p=128, u=2)
                stab = p1w.tile([128, TT, 2, HD], bf16, name="stab")
                stab_r = sina_d.rearrange("(tt p) (u d) -> p tt u d",
                                          p=128, u=2)
                # x/qkv-weight/table loads first; phase-2-only tensors after
                for wq in range(4):
                    sl = slice(wq * 4, (wq + 1) * 4)
                    nc.scalar.dma_start(out=wc_sb[:, sl], in_=wc_d[:, sl])
                nc.scalar.dma_start(out=ctab, in_=ctab_r)
                nc.scalar.dma_start(out=stab, in_=stab_r)
                # constants / aux columns after the DMA triggers
                make_identity(nc, ident)
                nc.vector.memset(eps_t, EPS * G * G)
                nc.vector.memset(ebias_t, -ESH * float(np.log(2.0)))
                nc.vector.memset(g_t, 1.0 / G)
                nc.gpsimd.memset(v8a[:, :, 64:128], 0.0)
                nc.gpsimd.memset(v8a[:, :, 64:65], 1.0)
                nc.vector.memset(v8b[:, :, 0:64], 0.0)
                nc.vector.memset(v8b[:, :, 32:33], 1.0)
                nc.gpsimd.memset(vr8a[:, :, 64:128], 0.0)
                nc.vector.memset(vr8b[:, :, 0:64], 0.0)
                nc.gpsimd.memset(vba[:, :, 64:68], 0.0)
                nc.gpsimd.memset(vba[:, :, 64:65], 1.0)
                nc.vector.memset(vbb[:, :, 0:64], 0.0)
                nc.vector.memset(vbb[:, :, 32:33], 1.0)

                pending_qb = []

                def flush_qb():
                    for qb_p, tt_p in pending_qb:
                        psqt = ps1c.tile([64, 5, 128], bf16, tag="psqt")
                        for u in range(5):
                            nc.tensor.transpose(psqt[:, u, :], qb_p[:, u, :],
                                                ident)
                        nc.scalar.copy(
                            qkT[:, :, tt_p * 128:(tt_p + 1) * 128], psqt
                        )
                    pending_qb.clear()

                def emit_v_batch(g):
                    # batched fp8/bf16 v staging for t-tiles 4g..4g+3
                    ts4 = slice(4 * g, 4 * g + 4)
                    vf = qkv_sp[:, ts4, NQK:NQKV]
                    nc.gpsimd.tensor_copy(v8a[:, ts4, 0:64], vf)
                    nc.gpsimd.tensor_copy(v8b[:, ts4, 64:128], vf)
                    nc.gpsimd.tensor_sub(vr8a[:, ts4, 0:64], vf,
                                         v8a[:, ts4, 0:64])
                    nc.gpsimd.tensor_copy(vr8b[:, ts4, 64:128],
                                          vr8a[:, ts4, 0:64])
                    nc.gpsimd.tensor_copy(vba[:, ts4, 0:64], vf)
                    nc.gpsimd.tensor_copy(vbb[:, ts4, 64:128], vf)

                for tt in range(TT):
                    xcr = p1x.tile([128, CC, 2, 128], f8, tag="xcr")
                    nc.sync.dma_start(out=xcr, in_=xc_d[:, tt])
                    psqkv = ps1b.tile([128, NQKV], f32, tag="psqkv")
                    # transposes of the PREVIOUS tt go first so the PE
                    # never waits on the rope chain
                    flush_qb()
                    # main: x8 (slot1) x W8 (slot0), cc-pairs as DR groups
                    for c2 in range(CC // 2):
                        mmr(psqkv, xcr[:, 2 * c2:2 * c2 + 2, 1, :],
                            wc_sb[:, 2 * c2:2 * c2 + 2, 0, :],
                            start=(c2 == 0), stop=False, perf_mode=DR)
                    # corr: (xr8, x8) x (W8, Wr8) = xr8.W8 + x8.Wr8
                    for cc in range(CC):
                        mmr(psqkv, xcr[:, cc, :, :], wc_sb[:, cc, :, :],
                            start=False, stop=(cc == CC - 1), perf_mode=DR)

                    # single fast staging copy frees the psum ring quickly
                    nc.vector.tensor_copy(qkv_sp[:, tt], psqkv)
                    qk5 = qkv_sp[:, tt, 0:NQK].rearrange(
                        "p (u d) -> p u d", u=5)

                    # rope FIRST on the raw (G-scaled) projections, the
                    # rms-norm scalar lands at the end: rope commutes with
                    # the per-(token,unit) rinv, so the sqrt chain computes
                    # concurrently instead of gating the whole chain
                    sq = p1t.tile([128, 5, HD], f32, tag="sq")
                    nc.scalar.activation(sq, qk5, AF.Square)
                    ssum = p1t.tile([128, 5, 1], f32, tag="ssum")
                    nc.vector.reduce_sum(ssum, sq, axis=AX.X)
                    rinv = p1t.tile([128, 5, 1], f32, tag="rinv")
                    nc.scalar.activation(rinv, ssum, AF.Sqrt,
                                         bias=eps_t[:, 0:1], scale=1.0 / HD)
                    nc.vector.reciprocal(rinv, rinv)
                    cq = ctab[:, tt, 0:1, :].to_broadcast([128, 4, HD])
                    qr = p1t.tile([128, 5, HD], bf16, tag="qr")
                    nc.vector.tensor_mul(qr[:, 0:4, :], qk5[:, 0:4, :], cq)
                    nc.vector.tensor_mul(qr[:, 4:5, :], qk5[:, 4:5, :],
                                         ctab[:, tt, 1:2, :])
                    qrot = p1t.tile([128, 5, HD], bf16, tag="qrot")
                    nc.gpsimd.tensor_mul(
                        qrot[:, 0:4, 0:32], qk5[:, 0:4, 32:64],
                        stab[:, tt, 0:1, 0:32].to_broadcast([128, 4, 32]),
                    )
                    nc.gpsimd.tensor_mul(
                        qrot[:, 0:4, 32:64], qk5[:, 0:4, 0:32],
                        stab[:, tt, 0:1, 32:64].to_broadcast([128, 4, 32]),
                    )
                    nc.gpsimd.tensor_mul(
                        qrot[:, 4:5, 0:32], qk5[:, 4:5, 32:64],
                        stab[:, tt, 1:2, 0:32],
                    )
                    nc.gpsimd.tensor_mul(
                        qrot[:, 4:5, 32:64], qk5[:, 4:5, 0:32],
                        stab[:, tt, 1:2, 32:64],
                    )
                    qa = p1t.tile([128, 5, HD], bf16, tag="qa")
                    nc.vector.tensor_add(qa, qr, qrot)
                    qb = p1t.tile([128, 5, HD], bf16, tag="qb")
                    nc.vector.tensor_mul(qb, qa,
                                         rinv.to_broadcast([128, 5, HD]))
                    pending_qb.append((qb, tt))
                    if tt % 4 == 3:
                        emit_v_batch(tt // 4)
                    if tt == 10:
                        # phase-2-only tensors ride in after the x stream
                        # has mostly landed (the 360GB/s DMA roof gates
                        # phase 1)
                        nc.scalar.dma_start(out=woc_sb, in_=woc_d[:])
                        nc.scalar.dma_start(
                            out=mpat_sb, in_=mpat_d.rearrange("n p f -> p n f")
                        )
                flush_qb()

            # ---------- phase 2: attention + den + out-proj ----------
            with (
                tc.tile_pool(name="p2e8", bufs=14) as p2e8,
                tc.tile_pool(name="p2eb", bufs=7) as p2eb,
                tc.tile_pool(name="p2e", bufs=4) as p2e,
                tc.tile_pool(name="p2o", bufs=2) as p2o,
                tc.tile_pool(name="ps2s", bufs=3, space="PSUM") as ps2s,
                tc.tile_pool(name="ps2c", bufs=2, space="PSUM") as ps2c,
            ):
                inv_sqrt_d = float(1.0 / np.sqrt(HD))
                ot_n = [0]
                act_free = [False]

                def out_proj_quarter(ib, t4, tail=False):
                    # 3-term fp8 DR out-proj; psum slots come from the shared
                    # score ring, staging copies alternate DVE / ACT
                    tt = ib * 4 + t4
                    tw = slice(tt * 128, (tt + 1) * 128)
                    ot = p2o.tile([128, D], bf16, tag="ot")
                    if tail:
                        # the score ring is free once the last exps are in
                        # flight: 2 cb per 2-bank slot, wide staging copies
                        for half4 in range(2):
                            pso = ps2s.tile([128, 2, 512], f32, tag="pss")
                            for hcb in range(2):
                                cb = half4 * 2 + hcb
                                cw = slice(cb * 512, (cb + 1) * 512)
                                mmr(pso[:, hcb, :], ctxc8[:, :, 0, tw],
                                    woc_sb[:, :, 1, cw],
                                    start=True, stop=False, perf_mode=DR)
                                for p in range(2):
                                    mmr(pso[:, hcb, :], ctxc8[:, p, :, tw],
                                        woc_sb[:, p, :, cw],
                                        start=False, stop=(p == 1),
                                        perf_mode=DR)
                            eng = (nc.scalar.copy if ot_n[0] % 2 == 1
                                   else nc.vector.tensor_copy)
                            ot_n[0] += 1
                            eng(ot[:, half4 * 1024:(half4 + 1) * 1024]
                                .rearrange("p (a b) -> p a b", a=2), pso)
                    else:
                        for cb in range(4):
                            cw = slice(cb * 512, (cb + 1) * 512)
                            pso = ps2c.tile([128, 512], f32, tag="psc")
                            mmr(pso, ctxc8[:, :, 0, tw], woc_sb[:, :, 1, cw],
                                start=True, stop=False, perf_mode=DR)
                            for p in range(2):
                                mmr(pso, ctxc8[:, p, :, tw],
                                    woc_sb[:, p, :, cw],
                                    start=False, stop=(p == 1), perf_mode=DR)
                            eng = (nc.scalar.copy
                                   if (act_free[0] and ot_n[0] % 2 == 1)
                                   else nc.vector.tensor_copy)
                            ot_n[0] += 1
                            eng(ot[:, cw], pso)
                    nc.sync.dma_start(out=out_d[tt * 128:(tt + 1) * 128, :],
                                      in_=ot)

                def den_pair(ib, pair, pe, po, den_sb):
                    # reciprocal of both heads' denominators -> broadcast
                    # across partitions with K=1 matmuls (value 1/G folds
                    # away the v-side G prescale) -> normalize ctx straight
                    # from psum, emit fp8 ctx + fp8 residual for out-proj.
                    iw = slice(ib * 512, (ib + 1) * 512)
                    with nc.allow_low_precision(
                        reason="1/den in bf16: 0.4% on softmax scale is "
                               "well inside the 2e-2 tolerance"
                    ):
                        nc.vector.reciprocal(den_sb[32:33, :], po[32:33, :])
                    pdb = ps2s.tile([128, 2, 512], f32, tag="pss")
                    mmr(pdb[0:64, 0, :], g_t[64:65, :], den_sb[64:65, :],
                        start=True, stop=True)
                    mmr(pdb[64:128, 0, :], g_t[32:33, :], den_sb[32:33, :],
                        start=True, stop=True)
                    nc.vector.tensor_copy(dbc[pair][:, iw], pdb[:, 0, :])
                    ctx_n = p2e.tile([128, 512], bf16, tag="ctx_n")
                    nc.vector.tensor_mul(ctx_n[0:64, :], pe[0:64, :],
                                         dbc[pair][0:64, iw])
                    nc.vector.tensor_mul(ctx_n[64:128, :], po[64:128, :],
                                         dbc[pair][64:128, iw])
                    nc.gpsimd.tensor_copy(ctxc8[:, pair, 0, iw], ctx_n)
                    nc.gpsimd.tensor_sub(ctxc8[:, pair, 1, iw], ctx_n,
                                         ctxc8[:, pair, 0, iw])

                def emit_scores(h, ib):
                    iw = slice(ib * 512, (ib + 1) * 512)
                    pairs, diag = ib_pairs_diag[ib]
                    et8s = []
                    for (j0, j1) in pairs:
                        pss = ps2s.tile([128, 2, 512], f32, tag="pss")
                        for half, jb in enumerate((j0, j1)):
                            mmr(pss[:, half, :],
                                qkT[:, 4, jb * 128:(jb + 1) * 128],
                                qkT[:, h, iw],
                                start=True, stop=True)
                        et8 = p2e8.tile([128, 2, 512], f8, tag="et8")
                        nc.scalar.activation(et8, pss, AF.Exp,
                                             scale=inv_sqrt_d,
                                             bias=ebias_t[:, 0:1])
                        et8s.append(et8)
                    etbs = []
                    for n in range(0, len(diag), 2):
                        dgrp = diag[n:n + 2]
                        pss = ps2s.tile([128, 2, 512], f32, tag="pss")
                        pre_g = 512
                        for half, (jb, pat) in enumerate(dgrp):
                            pre = pat_prefix[pat] if pat is not None else 0
                            pre_g = min(pre_g, pre)
                            mmr(pss[:, half, pre:512],
                                qkT[:, 4, jb * 128:(jb + 1) * 128],
                                qkT[:, h, ib * 512 + pre:(ib + 1) * 512],
                                start=True, stop=True)
                        etb = p2eb.tile([128, 2, 512], bf16, tag="etb")
                        nc.scalar.activation(etb[:, :, pre_g:512],
                                             pss[:, :, pre_g:512],
                                             AF.Exp, scale=inv_sqrt_d,
                                             bias=ebias_t[:, 0:1])
                        meng = (nc.vector.tensor_mul if ib == 0
                                else nc.gpsimd.tensor_mul)
                        for half, (jb, pat) in enumerate(dgrp):
                            if pat is None:
                                continue
                            pre = pat_prefix[pat]
                            meng(
                                etb[:, half, pre:512],
                                etb[:, half, pre:512],
                                mpat_sb[:, pat, pre:512],
                            )
                        etbs.append((etb, dgrp))
                    return et8s, etbs

                psc_of = {}
                den_sb_of = {}
                wo_queue = []

                def emit_pv(h, ib, et8s, etbs):
                    pairs, diag = ib_pairs_diag[ib]
                    pair, sub = divmod(h, 2)
                    psc = ps2c.tile([128, 512], f32, tag="psc")
                    psc_of[h, ib] = psc
                    if sub == 0:
                        ctx_out = psc
                        ctx_bout = psc[0:68, :]
                        va8, vr8, vab = v8a, vr8a, vba
                    else:
                        ctx_out = psc
                        ctx_bout = psc
                        va8, vr8, vab = v8b, vr8b, vbb
                    n_mm = 2 * len(pairs) + len(diag)
                    k = 0
                    for n, (j0, j1) in enumerate(pairs):
                        b2 = j0 // 2
                        assert j1 == j0 + 1 and j0 % 2 == 0
                        for vv in (va8, vr8):
                            mmr(ctx_out, vv[:, 2 * b2:2 * b2 + 2, :],
                                et8s[n], start=(k == 0),
                                stop=(k == n_mm - 1), perf_mode=DR)
                            k += 1
                    for etb, dgrp in etbs:
                        for half, (jb, pat) in enumerate(dgrp):
                            pre = (pat_prefix[pat]
                                   if pat is not None else 0)
                            mmr(ctx_bout[:, pre:512], vab[:, jb, :],
                                etb[:, half, pre:512],
                                start=(k == 0), stop=(k == n_mm - 1))
                            k += 1
                    if sub == 0:
                        den_sb = p2e.tile([65, 512], bf16, tag="den_sb")
                        with nc.allow_low_precision(
                            reason="1/den in bf16, inside 2e-2 tol"
                        ):
                            nc.vector.reciprocal(den_sb[64:65, :],
                                                 psc[64:65, :])
                        den_sb_of[pair, ib] = den_sb
                    else:
                        den_pair(ib, pair, psc_of[h - 1, ib], psc,
                                 den_sb_of.pop((pair, ib)))
                        for _ in range(2):
                            if wo_queue:
                                out_proj_quarter(*wo_queue.pop(0))

                # software pipeline: scores/exps of unit n overlap the PV /
                # den / out-proj of unit n-1 so the in-order PE queue never
                # parks on an exp wait.  Unit order: ib0 zipped with ib3
                # (tiny + huge complement each other and cover the phase-1
                # seam), then ib2, then ib1.  Completed i-blocks enqueue
                # out-proj quarters, popped two per den event.
                # pair-granular order: even/odd heads of a pair stay
                # adjacent (the 2-slot psc ring frees at each den event)
                units = [(h, ib) for ib in (0, 2, 3, 1)
                         for h in range(H_LOC)]
                done_cnt = {}
                prev = None
                for n, (h, ib) in enumerate(units):
                    act_free[0] = n >= 13
                    ets = emit_scores(h, ib)
                    if prev is not None:
                        emit_pv(prev[0], prev[1], *prev[2])
                        pib = prev[1]
                        done_cnt[pib] = done_cnt.get(pib, 0) + 1
                        if done_cnt[pib] == H_LOC:
                            wo_queue.extend((pib, t4) for t4 in range(4))
                    prev = (h, ib, ets)
                act_free[0] = True
                emit_pv(prev[0], prev[1], *prev[2])
                wo_queue.extend((prev[1], t4) for t4 in range(4))
                while wo_queue:
                    out_proj_quarter(*wo_queue.pop(0), tail=True)

    _split_multiwaits(nc)
    return nc


_CACHE = {}


def _get_program(mask_key, status, n_pat, pat_prefix):
    if mask_key not in _CACHE:
        _CACHE[mask_key] = _build_program(status, n_pat, pat_prefix)
    return _CACHE[mask_key]


def _f8_pair(a):
    """Split a into (fp8 value, fp8 residual)."""
    hi = a.astype(F8)
    lo = (a - hi.astype(np.float32)).astype(F8)
    return hi, lo


def _prepare(x, mask, cos, sin, W_query, W_key, W_value, W_out,
             q_scale, k_scale):
    """Host-side prep: transpose+fp8-split x, fold scales into rope tables,
    shard + fp8-split weights, classify the mask."""
    cos = np.asarray(cos, dtype=np.float32)
    sin = np.asarray(sin, dtype=np.float32)
    W_query = np.asarray(W_query, dtype=np.float32)
    W_key = np.asarray(W_key, dtype=np.float32)
    W_value = np.asarray(W_value, dtype=np.float32)
    W_out = np.asarray(W_out, dtype=np.float32)
    q_scale = np.asarray(q_scale, dtype=np.float32)
    k_scale = np.asarray(k_scale, dtype=np.float32)
    mask = np.asarray(mask)

    xT = np.asarray(x, dtype=np.float32).reshape(T, D).T  # [D, T]
    x8, xr8 = _f8_pair(xT)
    # xc [128, TT, CC, 2, 128]: slot0 = xr8, slot1 = x8
    xv = np.stack([xr8, x8], axis=0).reshape(2, CC, 128, TT, 128)
    xc = np.ascontiguousarray(xv.transpose(2, 3, 1, 0, 4))

    # rope = qn*cos' + shuffle32(qn)*sin' with the rotate-half signs and the
    # post-norm q/k scales folded into the tables
    def tables(scale):
        perm = np.concatenate([scale[HD // 2:], scale[:HD // 2]])
        c = (cos * scale[None, :]).astype(np.float32)
        s = (sin * perm[None, :]).astype(np.float32)
        s[:, :HD // 2] *= -1.0
        return c, s

    cq, sq_t = tables(q_scale)
    ck, sk_t = tables(k_scale)
    cosa = np.ascontiguousarray(
        np.concatenate([cq, ck], axis=1).astype(BF16)
    )
    sina = np.ascontiguousarray(
        np.concatenate([sq_t, sk_t], axis=1).astype(BF16)
    )

    status, patterns, prefixes = _classify_mask(mask)
    nc = _get_program(mask.tobytes(), status, patterns.shape[0], prefixes)
    patterns_bf = patterns.astype(BF16)

    in_maps = []
    for c in range(N_CORES):
        qcols = slice(c * H_LOC * HD, (c + 1) * H_LOC * HD)
        kvcols = slice(c * HD, (c + 1) * HD)
        wqkv = np.concatenate(
            [W_query[:, qcols], W_key[:, kvcols], W_value[:, kvcols]], axis=1
        ) * G
        w8, wr8 = _f8_pair(wqkv)  # [D, 384]
        # wc [128, CC, 2, 384]: slot0 = W8, slot1 = Wr8
        wc = np.ascontiguousarray(
            np.stack([w8, wr8], axis=0).reshape(2, CC, 128, NQKV_HOST)
            .transpose(2, 1, 0, 3)
        )
        woG = W_out[qcols, :] * G  # [256, D]
        wo8, wor8 = _f8_pair(woG)
        # woc [128, 2(pair), 2(slot), D]: slot0 = wor8, slot1 = wo8
        woc = np.ascontiguousarray(
            np.stack([wor8, wo8], axis=0).reshape(2, 2, 128, D)
            .transpose(2, 1, 0, 3)
        )
        in_maps.append({
            "xc": xc,
            "wc": wc,
            "woc": woc,
            "cosa": cosa, "sina": sina,
            "mpat": patterns_bf,
        })
    return nc, in_maps


NQKV_HOST = H_LOC * HD + 2 * HD


def kernel(x, mask, cos, sin, W_query, W_key, W_value, W_out,
           q_scale, k_scale):
    out_dtype = np.asarray(x).dtype
    nc, in_maps = _prepare(x, mask, cos, sin, W_query, W_key, W_value,
                           W_out, q_scale, k_scale)

    from concourse.bass_utils import run_bass_kernel_spmd

    res = run_bass_kernel_spmd(nc, in_maps, list(range(N_CORES)))
    acc = res.results[0]["out"].astype(np.float32)
    for c in range(1, N_CORES):
        acc = acc + res.results[c]["out"].astype(np.float32)
    acc *= 1.0 / G  # compensate the W_out prescale
    return acc.reshape(1, T, D).astype(out_dtype)
